# revision 76
# baseline (speedup 1.0000x reference)
"""Trainium2 Bass kernel for nn_Attention (GQA + RoPE + sliding-window mask).

Sharding: tensor-parallel over heads across 8 cores. Each core gets 4 q heads
and exactly 1 kv head (32 q / 8 kv heads, GQA group = 4). The reference's
quirky output flatten ((H,S,D)->(H,D,S)->reshape(S, H*D)) makes the final
projection contract over (d-parity, sequence) instead of heads, so the final
output is row-sharded by head block: core c produces rows [256c, 256c+256) of
the (2048, 4096) result with NO collective at all.

Design notes (all on one NeuronCore, same program on all 8 = pure SPMD).
Phases are kept DENSE and separate: the CoreSim cost model resets the PE
p-state ramp on every idle gap (half clock for 3us after a gap), so a
fragmented PE stream loses more than interleaving gains.
  phase 1: QKV projections as 3-term fp8 DoubleRow matmuls
           (x_hi*w_hi + x_lo*w_hi + x_hi*w_lo at 0.5 cycles/row and 256-deep
           contraction per instruction = 3/4 the PE cost of bf16, and MORE
           accurate: hi+lo carries ~7 mantissa bits vs bf16's 8). Host
           stages fp8(16x) / fp8(64w) splits; 1/1024 is folded into the
           rope tables and the V-copy scale. RoPE on DVE, PE transposes
           into bf16 [d, s] layouts for Q^T / K^T; V kept [s, d].
  phase 2: TRANSPOSED scores S^T[k, q] = (K^T tile)^T @ Q^T (stationary K
           tile, moving Q) so the exp output P^T[k, q] feeds the PV matmul
           directly as the moving operand -- no P transpose at all. Softmax
           runs WITHOUT the max pass (logits here are bounded ~|20|, and
           exp(-1e9)=0 handles the mask), so exp reads PSUM scores directly.
           Head-PAIR blocks share one [P, 1024] score tile and ONE fused exp
           per key tile (halves ACT per-op overhead; ACT is the phase-2
           bottleneck). A pure 0/-inf mask is applied MULTIPLICATIVELY (0/1,
           bf16, DVE 2x) to P^T after the exp, off the matmul->exp critical
           path; PV is emitted one unit late so exp latency hides under the
           next unit's matmuls; per-block store segments are lagged into the
           next block so the exp stream never breaks. Denominators come from
           1-column ones-matmuls on the PE (cost ~zero: matmul cost is
           output-free-size only; the interp allows one live accumulation
           group per PSUM bank, so the 8 chains run sequentially),
           reciprocals on DVE, applied as ACT scale-copies when storing A
           (walrus rejects InstTensorScalarPtr, the DVE equivalent).
  phase 3: final projection as 3-term fp8 DoubleRow vs wo (host-split
           fp8(64*wo)); 16*A is split on-device into fp8 hi+residual. wo
           streams as 256-col chunks through a 6-deep ring; output DMAs
           issue from ACT because SP's queue is busy streaming wo.
           The first 6 row-tile-0 blocks are INJECTED between phase-2's
           second-half softmax blocks (row tile 0's A completes at the
           phase-2 midpoint): they fill phase 2's idle PE with dense DR
           work, evacuating via DVE+SP so the ACT exp stream is untouched,
           and their consumed wo ring slots re-stage on the idle DMA device.
           The 8th PSUM bank this needs comes from packing the denominator
           accumulators (bitcast f32 region) into the Ap-transpose bank.
"""

import numpy as np
from contextlib import ExitStack

P = 128
D = 128  # head dim
NH = 4   # q heads per core
CORES = 8
NEG_THRESH = -1e8


def build_attention_nc(SEQ, DIM, plan, n_uniq):
    """Build the per-core Bass program.

    plan: list over q-supers qs (SEQ//512 entries) of lists of
          (kt, uid, lead, w) over included 128-wide key tiles; uid == -1
          means no mask add, else index into the (transposed) maskb tensor,
          applied to cols [128*lead, 128*lead + w). lead = number of leading
          fully-masked 128-col spans (skipped in matmul/exp, zero-filled in
          P^T). kt absent = fully masked.
    """
    import concourse.bass as bass
    import concourse.bacc as bacc
    import concourse.mybir as mybir
    import concourse.tile as tile
    from concourse.masks import make_identity

    f32 = mybir.dt.float32
    bf16 = mybir.dt.bfloat16
    A_ = mybir.AluOpType
    AF = mybir.ActivationFunctionType

    ST = SEQ // P          # 16 s-tiles
    DD = DIM // P          # 32 contraction tiles
    QS = SEQ // 512        # 4 query supers
    JT = 2 * SEQ // P      # 32 j-tiles for final matmul
    MC = DIM // 512        # 8 output chunks
    ITILES = (NH * 64) // P  # 2 output row tiles
    EW = NH * D            # 512 q features per core
    assert NH == 4 and SEQ % 512 == 0 and DIM % 512 == 0

    nc = bacc.Bacc(trn_type="TRN2", debug=False, num_devices=CORES)

    e4 = mybir.dt.float8e4
    DR = mybir.MatmulPerfMode.DoubleRow
    # x^T tiled by dim-tile: x?[p, t, s] = fp8(16*x)[s, 128t+p], split into a
    # hi part and an fp8 residual (hi+lo carries ~7 mantissa bits, better
    # than bf16). The 16x/64x scales keep the residuals out of fp8-subnormal
    # territory; the combined 1024x is folded into the rope tables and the
    # V-copy scale. fp8 DoubleRow matmuls pair two 128-contraction tiles per
    # instruction at 0.5 cycles/row: the 3-term (hh + lh + hl) projection
    # runs at ~3/4 the PE cost of bf16. Layout contiguous in s so streamed
    # 512-token slices have 512B runs per partition (no DMA penalty).
    xh = nc.dram_tensor("xh", [P, DD, SEQ], e4, kind="ExternalInput").ap()
    xl = nc.dram_tensor("xl", [P, DD, SEQ], e4, kind="ExternalInput").ap()
    # packed projection weights [wq_c; wk_c; wv_c]: w?[p, t, f] ~ w[f, 128t+p]
    wh = nc.dram_tensor("wh", [P, DD, EW + 2 * D], e4, kind="ExternalInput").ap()
    wl = nc.dram_tensor("wl", [P, DD, EW + 2 * D], e4, kind="ExternalInput").ap()
    # rope tables [cos_rep | sin_rep] with sqrt(scale) folded in
    cs = nc.dram_tensor("cs", [SEQ, EW], f32, kind="ExternalInput").ap()
    # transposed partial mask blocks [k 128, q <=512], left-aligned;
    # bf16 0/1 factors when the mask is multiplicative, else f32 additive
    mul_mask = any(
        e[4] for row in plan for e in row if e[1] >= 0
    )
    mb = nc.dram_tensor(
        "maskb", [max(n_uniq, 1), P, 512], bf16 if mul_mask else f32,
        kind="ExternalInput",
    ).ap()
    # wo transposed, 64x-scaled, fp8 hi/lo split, pre-tiled [p, mc, jt, m] so
    # each 256-col chunk is one DMA with 8KB-contiguous per-partition runs
    MC2 = DIM // 256
    woh = nc.dram_tensor(
        "woh", [P, MC2, JT, 256], e4, kind="ExternalInput"
    ).ap()
    wol = nc.dram_tensor(
        "wol", [P, MC2, JT, 256], e4, kind="ExternalInput"
    ).ap()
    out = nc.dram_tensor("out", [NH * 64, DIM], f32, kind="ExternalOutput").ap()

    with tile.TileContext(nc) as tc, ExitStack() as ctx:
        const = ctx.enter_context(tc.tile_pool(name="const", bufs=1))
        idP = const.tile([P, P], bf16)
        make_identity(nc, idP)
        # 1/16 folds the fp8 A-staging scale into the softmax denominators:
        # dpt = sum(P)/16 -> recip = 16/sum -> Aall holds 16*A (fp8-friendly)
        ones = const.tile([P, 1], bf16)
        nc.vector.memset(ones, 1.0 / 16.0)
        # 1/1024 row for DVE-side output descaling (plain TT is walrus-safe)
        sc1024 = const.tile([P, 256], f32)
        nc.vector.memset(sc1024, 1.0 / 1024.0)

        pers = ctx.enter_context(tc.tile_pool(name="pers", bufs=1))
        QT = pers.tile([P, NH, SEQ], bf16)   # [d, h, s]
        KT = pers.tile([P, SEQ], bf16)       # [d, s]
        Vt = pers.tile([P, ST, D], bf16)     # [s(part), stile, d]
        if n_uniq > 0:
            mbt = pers.tile([P, n_uniq, 512], bf16 if mul_mask else f32)
        Aall = [
            pers.tile([P, 2 * ST * D], bf16, name=f"Aall{i}")
            for i in range(ITILES)
        ]
        A8h = [
            pers.tile([P, 2 * ST * D], e4, name=f"A8h{i}")
            for i in range(ITILES)
        ]
        A8l = [
            pers.tile([P, 2 * ST * D], e4, name=f"A8l{i}")
            for i in range(ITILES)
        ]
        stat = ctx.enter_context(tc.tile_pool(name="stat", bufs=16))

        # ---------------- phase 1 ----------------
        with (
            tc.tile_pool(name="wpool", bufs=1) as wpool,
            tc.tile_pool(name="xpool", bufs=2) as xpool,
            tc.tile_pool(name="cspool", bufs=4) as cspool,
            tc.tile_pool(name="rpool", bufs=2) as rpool,
            tc.tile_pool(name="qps", bufs=2, space="PSUM") as qps,
            tc.tile_pool(name="kvps", bufs=2, space="PSUM") as kvps,
            tc.tile_pool(name="tps", bufs=2, space="PSUM") as tps,
        ):
            # startup-latency-aware DMA order: the first (hi*hi) matmuls need
            # only wh t-tiles 0..3 and the first half of xh s-group 0; the
            # hi*lo and lo*hi terms follow, so wl/xl stream behind
            wht = wpool.tile([P, DD, EW + 2 * D], e4, name="wht")
            wlt = wpool.tile([P, DD, EW + 2 * D], e4, name="wlt")
            xs0h = xpool.tile([P, DD, 512], e4, tag="xsh", name="xs0h")
            xs0l = xpool.tile([P, DD, 512], e4, tag="xsl", name="xs0l")
            # supply order tracks the term order hh -> hl -> lh of s-group 0,
            # in fine-grained chunks so the first matmuls start after ~2 DMAs
            cst0 = [cspool.tile([P, EW], f32, tag="cs", name=f"cst{l}")
                    for l in range(4)]
            for g in range(8):
                nc.sync.dma_start(
                    out=wht[:, 4 * g : 4 * g + 4, :],
                    in_=wh[:, 4 * g : 4 * g + 4, :],
                )
                nc.sync.dma_start(
                    out=xs0h[:, 4 * g : 4 * g + 4, :],
                    in_=xh[:, 4 * g : 4 * g + 4, 0:512],
                )
                if g == 1:
                    for l in range(2):
                        nc.sync.dma_start(
                            out=cst0[l], in_=cs[l * P : (l + 1) * P, :]
                        )
            for g in range(8):
                nc.sync.dma_start(
                    out=wlt[:, 4 * g : 4 * g + 4, :],
                    in_=wl[:, 4 * g : 4 * g + 4, :],
                )
                nc.sync.dma_start(
                    out=xs0l[:, 4 * g : 4 * g + 4, :],
                    in_=xl[:, 4 * g : 4 * g + 4, 0:512],
                )
                if g == 1:
                    for l in range(2, 4):
                        nc.sync.dma_start(
                            out=cst0[l], in_=cs[l * P : (l + 1) * P, :]
                        )
            def finish_stile(st, cst, Qp, KVp):
                # rope via strided even/odd halves; final add/sub writes
                # bf16 so the PE transposes below run at 1 cycle/row
                HF = EW // 2
                rq = rpool.tile([P, EW], bf16, tag="rq")
                rk = rpool.tile([P, D], bf16, tag="rk")
                t1 = rpool.tile([P, HF], f32, tag="t1")
                t2 = rpool.tile([P, HF], f32, tag="t2")
                q_ev, q_od = Qp[:, 0:EW:2], Qp[:, 1:EW:2]
                cosr, sinr = cst[:, 0:HF], cst[:, HF : 2 * HF]
                nc.vector.tensor_tensor(out=t1, in0=q_ev, in1=cosr, op=A_.mult)
                nc.vector.tensor_tensor(out=t2, in0=q_od, in1=sinr, op=A_.mult)
                nc.vector.tensor_tensor(
                    out=rq[:, 0:EW:2], in0=t1, in1=t2, op=A_.subtract
                )
                nc.vector.tensor_tensor(out=t1, in0=q_ev, in1=sinr, op=A_.mult)
                nc.vector.tensor_tensor(out=t2, in0=q_od, in1=cosr, op=A_.mult)
                nc.vector.tensor_tensor(
                    out=rq[:, 1:EW:2], in0=t1, in1=t2, op=A_.add
                )

                k_ev, k_od = KVp[:, 0:D:2], KVp[:, 1:D:2]
                cosk, sink = cst[:, 0 : D // 2], cst[:, HF : HF + D // 2]
                t1k, t2k = t1[:, 0 : D // 2], t2[:, 0 : D // 2]
                nc.vector.tensor_tensor(out=t1k, in0=k_ev, in1=cosk, op=A_.mult)
                nc.vector.tensor_tensor(out=t2k, in0=k_od, in1=sink, op=A_.mult)
                nc.vector.tensor_tensor(
                    out=rk[:, 0:D:2], in0=t1k, in1=t2k, op=A_.subtract
                )
                nc.vector.tensor_tensor(out=t1k, in0=k_ev, in1=sink, op=A_.mult)
                nc.vector.tensor_tensor(out=t2k, in0=k_od, in1=cosk, op=A_.mult)
                nc.vector.tensor_tensor(
                    out=rk[:, 1:D:2], in0=t1k, in1=t2k, op=A_.add
                )

                # V -> bf16 [s, d] (ACT copy, cast; 1/1024 undoes the
                # fp8 staging scales 16x on x and 64x on w)
                nc.scalar.activation(
                    out=Vt[:, st, :], in_=KVp[:, D : 2 * D], func=AF.Copy,
                    scale=1.0 / 1024.0,
                )

                # transpose rq (per head) and rk into bf16 [d, s] layouts
                Tt = tps.tile([P, EW + D], bf16, tag="T")
                for h in range(NH):
                    nc.tensor.transpose(
                        Tt[:, h * P : (h + 1) * P], rq[:, h * P : (h + 1) * P],
                        idP,
                    )
                nc.tensor.transpose(Tt[:, EW : EW + D], rk, idP)
                nc.scalar.activation(
                    out=QT[:, :, st * P : (st + 1) * P],
                    in_=Tt[:, 0:EW].rearrange("p (h s) -> p h s", h=NH),
                    func=AF.Copy,
                )
                nc.scalar.activation(
                    out=KT[:, st * P : (st + 1) * P],
                    in_=Tt[:, EW : EW + D],
                    func=AF.Copy,
                )

            SG = ST // 4  # stream x in 4 groups of 4 s-tiles
            for sg in range(SG):
                if sg == 0:
                    xsh, xsl = xs0h, xs0l
                    csts = cst0
                else:
                    xsh = xpool.tile([P, DD, 512], e4, tag="xsh")
                    nc.sync.dma_start(
                        out=xsh, in_=xh[:, :, sg * 512 : (sg + 1) * 512]
                    )
                    xsl = xpool.tile([P, DD, 512], e4, tag="xsl")
                    nc.sync.dma_start(
                        out=xsl, in_=xl[:, :, sg * 512 : (sg + 1) * 512]
                    )
                    csts = []
                    for l in range(4):
                        st = 4 * sg + l
                        c = cspool.tile([P, EW], f32, tag="cs")
                        nc.sync.dma_start(
                            out=c, in_=cs[st * P : (st + 1) * P, :]
                        )
                        csts.append(c)
                TP = DD // 2
                terms = [(xsh, wht), (xsh, wlt), (xsl, wht)]

                def emit_mm(ls, tiles):
                    # emit the 3-term DR chains for s-tiles `ls`, interleaved
                    # tp-major so demand tracks the chunked DMA supply order;
                    # each s-tile's rope/store is emitted as soon as ITS
                    # chain closes so the psum bufs recycle early
                    for term, (xst, wt) in enumerate(terms):
                        for tp in range(TP):
                            for i, l in enumerate(ls):
                                Qp, KVp = tiles[i]
                                lhsT = xst[:, 2 * tp : 2 * tp + 2,
                                           l * P : (l + 1) * P]
                                first = term == 0 and tp == 0
                                last = term == 2 and tp == TP - 1
                                nc.tensor.matmul(
                                    Qp, lhsT,
                                    wt[:, 2 * tp : 2 * tp + 2, 0:EW],
                                    start=first, stop=last, perf_mode=DR,
                                )
                                nc.tensor.matmul(
                                    KVp, lhsT,
                                    wt[:, 2 * tp : 2 * tp + 2,
                                       EW : EW + 2 * D],
                                    start=first, stop=last, perf_mode=DR,
                                )
                            if term == 2 and tp == TP - 1:
                                for i, l in enumerate(ls):
                                    finish_stile(4 * sg + l, csts[l],
                                                 *tiles[i])

                for ls in [(0,), (1,), (2,), (3,)]:
                    tiles = [
                        (qps.tile([P, EW], f32, tag="Qp", name=f"Qp{l}"),
                         kvps.tile([P, 2 * D], f32, tag="KVp", name=f"KVp{l}"))
                        for l in ls
                    ]
                    emit_mm(ls, tiles)
        # ---------------- phase 2: head-pair blocks ----------------
        # per unit (key tile kt): scores for BOTH heads of the pair into one
        # [P, 1024] psum tile, ONE fused exp over both halves, deferred PV.
        # Each block's store segment (recips / A store) is emitted LAGGED,
        # after the next block's first unit, so the ACT exp stream never
        # breaks at block boundaries.
        wopool = ctx.enter_context(tc.tile_pool(name="wopool", bufs=6))
        osb = ctx.enter_context(tc.tile_pool(name="osb", bufs=2))
        with (
            tc.tile_pool(name="ptpool", bufs=20) as ptpool,
            tc.tile_pool(name="atsb", bufs=4) as atsb,
            tc.tile_pool(name="sps", bufs=2, space="PSUM") as sps,
            tc.tile_pool(name="atps", bufs=2, space="PSUM") as atps,
            # one [P, 1024] bf16 bank carries BOTH the Ap transposes (cols
            # 0:512) and, bitcast to f32, the denominator accumulators (cols
            # 512:528) -- PSUM slots are bank-granular, and packing them
            # frees the 8th bank for the ops_inj pool below
            tc.tile_pool(name="aps", bufs=1, space="PSUM") as aps,
            tc.tile_pool(name="ops_inj", bufs=1, space="PSUM") as ops_inj,
        ):
            # wo prefetch (SP queue position: right after phase-1 DMAs; the
            # DMA device is idle during phase 2). 256-wide chunks, hi+lo fp8.
            wots = {}

            def stage_wo(mc):
                th = wopool.tile([P, JT, 256], e4, tag="woh", name=f"woh{mc}")
                tl = wopool.tile([P, JT, 256], e4, tag="wol", name=f"wol{mc}")
                nc.sync.dma_start(out=th, in_=woh[:, mc, :, :])
                nc.sync.dma_start(out=tl, in_=wol[:, mc, :, :])
                wots[mc] = (th, tl)

            if n_uniq > 0:
                nc.sync.dma_start(out=mbt, in_=mb.rearrange("u p m -> p u m"))
            for mc in range(6):
                stage_wo(mc)

            def pair_block_gen(hp, qs):
                """hp in {0,1}: heads (2hp, 2hp+1). Yields per unit; the
                final segment (after last yield) is the block tail."""
                row = plan[qs]
                h0 = 2 * hp
                if not row:
                    for qi in range(4):
                        i = 4 * qs + qi
                        nc.vector.memset(
                            Aall[hp][:, i * 2 * P : (i + 1) * 2 * P], 0.0
                        )
                    yield
                    return
                Ats = [atps.tile([P, 512], f32, tag="At", name=f"At{hl}")
                       for hl in range(2)]
                mt = aps.tile([P, 1024], bf16, tag="Ap")
                Ap = mt[:, 0:512]
                dpt = mt[:, 512:528].bitcast(f32)  # [P, 8] f32 region
                pts = []
                pend = None  # deferred PV (one unit of lag hides exp latency)
                for n, (kt, uid, lead, w, mul) in enumerate(row):
                    if pend is not None:
                        pkt, pPT, pn = pend
                        for hl in range(2):
                            nc.tensor.matmul(
                                Ats[hl], Vt[:, pkt, :],
                                pPT[:, hl * 512 : (hl + 1) * 512],
                                start=(pn == 0), stop=False,
                            )
                    off = lead * P
                    S = sps.tile([P, 1024], f32, tag="S")
                    for hl in range(2):
                        nc.tensor.matmul(
                            S[:, hl * 512 + off : (hl + 1) * 512],
                            KT[:, kt * P : (kt + 1) * P],
                            QT[:, h0 + hl, qs * 512 + off : (qs + 1) * 512],
                            start=True, stop=True,
                        )
                        if uid >= 0 and not mul:
                            # additive mask must run pre-exp (ACT waits on it)
                            sl = S[:, hl * 512 + off : hl * 512 + off + w]
                            nc.vector.tensor_add(sl, sl, mbt[:, uid, 0:w])
                    PT = ptpool.tile([P, 1024], bf16, tag="PT")
                    sv = S.rearrange("p (hl q) -> p hl q", hl=2)[:, :, off:512]
                    pv = PT.rearrange("p (hl q) -> p hl q", hl=2)[:, :, off:512]
                    nc.scalar.activation(out=pv, in_=sv, func=AF.Exp)
                    if uid >= 0 and mul:
                        # 0/1 mask applied to P post-exp (all-bf16 -> DVE 2x
                        # mode; the deferred PV gives it a unit of slack)
                        for hl in range(2):
                            sl = PT[:, hl * 512 + off : hl * 512 + off + w]
                            nc.vector.tensor_tensor(
                                out=sl, in0=sl, in1=mbt[:, uid, 0:w],
                                op=A_.mult,
                            )
                    if off:
                        for hl in range(2):
                            nc.vector.memset(
                                PT[:, hl * 512 : hl * 512 + off], 0.0
                            )
                    pts.append(PT)
                    pend = (kt, PT, n)
                    yield
                pkt, pPT, pn = pend
                for hl in range(2):
                    nc.tensor.matmul(
                        Ats[hl], Vt[:, pkt, :],
                        pPT[:, hl * 512 : (hl + 1) * 512],
                        start=(pn == 0), stop=True,
                    )
                # denominators via 1-wide ones-matmuls (the interp models one
                # live accumulation group per PSUM bank, so the 8 chains run
                # one after another). Emitted in-block; the store segment
                # below is lagged into the next block.
                for hl in range(2):
                    for qi in range(4):
                        col = hl * 4 + qi
                        for n, PT in enumerate(pts):
                            nc.tensor.matmul(
                                dpt[:, col : col + 1],
                                PT[:, hl * 512 + qi * P : hl * 512 + (qi + 1) * P],
                                ones,
                                start=(n == 0), stop=(n == len(pts) - 1),
                                skip_group_check=True,
                            )
                yield  # --- lag point: store segment pulled later ---
                for hl in range(2):
                    recips = []
                    for qi in range(4):
                        col = hl * 4 + qi
                        r = stat.tile([P, 1], f32, tag="recip")
                        nc.vector.reciprocal(r, dpt[:, col : col + 1])
                        recips.append(r)
                    Atsb = atsb.tile([P, 512], bf16, tag="Atsb")
                    nc.vector.tensor_copy(out=Atsb, in_=Ats[hl])
                    for qi in range(4):
                        nc.tensor.transpose(
                            Ap[:, qi * P : (qi + 1) * P],
                            Atsb[:, qi * P : (qi + 1) * P],
                            idP,
                        )
                    # Aall layout: [sp, (t*2 + dd)*128 + hb*64 + p] so the
                    # final matmul's stationary slices are contiguous
                    Ah = Aall[hp]
                    for qi in range(4):
                        i = 4 * qs + qi
                        dview = Ah[:, i * 2 * P : (i + 1) * 2 * P].rearrange(
                            "a (dd j) -> a dd j", dd=2
                        )[:, :, hl * 64 : hl * 64 + 64].rearrange(
                            "a dd p -> a p dd"
                        )
                        src = Ap[:, qi * P : (qi + 1) * P].rearrange(
                            "a (p two) -> a p two", two=2
                        )
                        # per-partition scale-copy on ACT (walrus rejects the
                        # DVE InstTensorScalarPtr equivalent)
                        nc.scalar.activation(
                            out=dview, in_=src, func=AF.Copy,
                            scale=recips[qi],
                        )

            def p3_gen(it, mc, opool, on_act):
                """Yield after each of the 3 DR terms so injection can feed
                the PE in ~0.9us segments without starving the ACT exps."""
                woth, wotl = wots[mc]
                O = opool.tile([P, 256], f32, tag="O")
                # 3-term fp8 DoubleRow: (Ah+Al)@(wh+wl) minus the Al@wl
                # term; each DR instruction contracts a PAIR of j-tiles
                terms = [(A8h[it], woth), (A8h[it], wotl), (A8l[it], woth)]
                for term, (Av, wv) in enumerate(terms):
                    Av4 = Av.rearrange("p (t d j) -> p t d j", d=2, j=P)
                    for ddj in range(2):
                        for u in range(ST // 2):
                            first = term == 0 and ddj == 0 and u == 0
                            last = (term == 2 and ddj == 1
                                    and u == ST // 2 - 1)
                            nc.tensor.matmul(
                                O,
                                Av4[:, 2 * u : 2 * u + 2, ddj, :],
                                wv[:, ddj * ST + 2 * u
                                   : ddj * ST + 2 * u + 2, :],
                                start=first, stop=last, perf_mode=DR,
                            )
                    if term < 2:
                        yield
                Ot = osb.tile([P, 256], f32, tag="Ot")
                # 1/1024 undoes the 16x (A) and 64x (wo) fp8 scales; during
                # phase 2 the evacuation runs on DVE (ACT is the softmax
                # bottleneck), in the tail on ACT (SP busy streaming wo)
                if on_act:
                    nc.scalar.activation(
                        out=Ot, in_=O, func=AF.Copy, scale=1.0 / 1024.0
                    )
                    nc.scalar.dma_start(
                        out=out[it * P : (it + 1) * P,
                                mc * 256 : (mc + 1) * 256],
                        in_=Ot,
                    )
                else:
                    nc.vector.tensor_tensor(
                        out=Ot, in0=O, in1=sc1024, op=A_.mult
                    )
                    nc.sync.dma_start(
                        out=out[it * P : (it + 1) * P,
                                mc * 256 : (mc + 1) * 256],
                        in_=Ot,
                    )

            def p3_block(it, mc, opool, on_act):
                for _ in p3_gen(it, mc, opool, on_act):
                    pass

            def split_a8(it, chunks=4, on_act=True):
                # split 16*A (bf16) into fp8 hi + residual
                for ch in range(chunks):
                    sl = slice(ch * (2 * ST * D // chunks),
                               (ch + 1) * (2 * ST * D // chunks))
                    if on_act:
                        nc.scalar.activation(
                            out=A8h[it][:, sl], in_=Aall[it][:, sl],
                            func=AF.Copy,
                        )
                    else:
                        nc.vector.tensor_copy(
                            out=A8h[it][:, sl], in_=Aall[it][:, sl]
                        )
                    nc.vector.tensor_tensor(
                        out=A8l[it][:, sl], in0=Aall[it][:, sl],
                        in1=A8h[it][:, sl], op=A_.subtract,
                    )

            # drive blocks hp-major; pull the previous block's store segment
            # right after the current block's FIRST unit (before the current
            # block's first PV touches the recycled At banks). Once hp0's A
            # is complete, early it0 output-projection blocks are injected
            # one DR TERM per softmax unit: the PE fills its idle time with
            # dense work while the ACT exp stream stays fed, all injected
            # evacuation off the ACT critical path. Consumed wo ring slots
            # are immediately re-staged on the idle phase-2 DMA device.
            N_INJ = 6
            inj = {"gen": None, "mc": 0, "stage": [0, 1, 2, 3, 4, 5]}

            def pump_inj():
                if inj["gen"] is None:
                    if inj["mc"] >= N_INJ:
                        return
                    inj["gen"] = p3_gen(0, inj["mc"], ops_inj, on_act=False)
                try:
                    next(inj["gen"])
                except StopIteration:
                    inj["gen"] = None
                    inj["mc"] += 1
                    if inj["stage"]:
                        stage_wo(inj["stage"].pop(0))

            pending = None
            for hp in range(2):
                for qs in range(QS):
                    g = pair_block_gen(hp, qs)
                    L = len(plan[qs])
                    nunits = max(1, L)
                    for n in range(nunits + (1 if L else 0)):
                        next(g)  # units, then the denominator segment
                        if n == 0 and pending is not None:
                            try:
                                next(pending)
                            except StopIteration:
                                pass
                            pending = None
                            if hp == 1 and qs == 0:
                                # hp0's last store just emitted: A of row
                                # tile 0 is complete -> split it on DVE
                                split_a8(0, on_act=False)
                    if pending is not None:  # empty-row block: flush now
                        try:
                            next(pending)
                        except StopIteration:
                            pass
                    pending = g if L else None
                    if hp == 1 and qs >= 1:
                        for _ in range(6):  # two whole blocks, term-paced
                            pump_inj()
            if pending is not None:
                try:
                    next(pending)
                except StopIteration:
                    pass
            # it1's A is now complete: split it on DVE under the remaining
            # injected blocks so the tail starts unblocked
            split_a8(1, on_act=False)
            while inj["mc"] < N_INJ:
                pump_inj()

        # ---------------- phase 3 tail ----------------
        # chunks 6..9 were consumed by injected it0 blocks and not yet
        # re-staged; chunks 10..15 get their first staging inside the loop
        for mc in range(6, N_INJ):
            stage_wo(mc)
        with tc.tile_pool(name="ops", bufs=4, space="PSUM") as ops:
            for mc in range(MC2):
                if N_INJ <= mc + 6 < MC2:
                    stage_wo(mc + 6)
                for it in range(ITILES):
                    if it == 0 and mc < N_INJ:
                        continue  # done during phase 2
                    p3_block(it, mc, ops, on_act=True)

    nc.compile()
    return nc


def analyze_mask(mask, SEQ):
    """Classify transposed 128-key x 512-query mask blocks per (kt, qs):
    skip / free / masked(dedup uid). Partial blocks store only the masked
    window: lead = # of leading fully-masked 128-col spans, w = width of the
    remaining span that contains any masked element. Blocks stored TRANSPOSED
    [k, q], left-aligned into a [128, 512] buffer."""
    QS = SEQ // 512
    KTOT = SEQ // P
    uniq = {}
    blocks = []
    plan = []
    # every query row needs at least one allowed key (no-max softmax would
    # otherwise divide by zero; the reference's uniform-distribution quirk
    # for fully-masked rows is not representable in this fast path)
    assert (mask > NEG_THRESH).any(axis=1).all(), "fully masked query row"
    # a pure 0/-inf mask can be applied MULTIPLICATIVELY (0/1) to P after the
    # exp, off the matmul->exp critical path; finite-valued masks must stay
    # additive pre-exp
    mul = bool(((mask <= NEG_THRESH) | (mask == 0.0)).all())
    for qs in range(QS):
        row = []
        for kt in range(KTOT):
            blk = mask[qs * 512 : (qs + 1) * 512, kt * P : (kt + 1) * P]
            if (blk <= NEG_THRESH).all():
                continue
            if not blk.any():
                row.append((kt, -1, 0, 0, mul))
                continue
            bT = np.ascontiguousarray(blk.T)  # [128 k, 512 q]
            col_all = (bT <= NEG_THRESH).all(axis=0)
            col_any = bT.any(axis=0)
            lead = 0
            while lead < 3 and col_all[lead * P : (lead + 1) * P].all():
                lead += 1
            last_any = int(np.nonzero(col_any)[0].max())
            w = (last_any // P + 1) * P - lead * P
            sl = bT[:, lead * P : lead * P + w]
            key = (w, sl.tobytes())
            if key not in uniq:
                uniq[key] = len(blocks)
                buf = np.zeros((P, 512), np.float32)
                buf[:, 0:w] = (sl > NEG_THRESH) if mul else sl
                blocks.append(buf)
            row.append((kt, uniq[key], lead, w, mul))
        plan.append(row)
    return plan, blocks


def make_rope_tables(cos_freq, sin_freq, SEQ, scale_quarter):
    """[cos_rep (SEQ, NH*64) | sin_rep (SEQ, NH*64)], sqrt(scale) folded in."""
    cos_t = np.tile(np.asarray(cos_freq, np.float32) * scale_quarter, (1, NH))
    sin_t = np.tile(np.asarray(sin_freq, np.float32) * scale_quarter, (1, NH))
    return np.ascontiguousarray(
        np.concatenate([cos_t, sin_t], axis=1).astype(np.float32)
    )


def stage_in_maps(x, cos_freq, sin_freq, wq, wk, wv, wo, plan, blocks):
    """Host-side input staging (shared by kernel() and test harnesses)."""
    import ml_dtypes

    bf16 = ml_dtypes.bfloat16
    e4 = ml_dtypes.float8_e4m3
    SEQ, DIM = x.shape
    DD = DIM // P
    n_uniq = len(blocks)
    # rope tables fold sqrt(scale) per side AND 1/1024 (fp8 staging scales)
    scale_quarter = np.float32(D ** -0.25) / np.float32(1024.0)
    cs = make_rope_tables(cos_freq, sin_freq, SEQ, scale_quarter)

    def tile_x(a):
        # [p, t, s] = a[s, 128t+p]
        return np.ascontiguousarray(a.reshape(SEQ, DD, P).transpose(2, 1, 0))

    xs16 = 16.0 * np.asarray(x, np.float32)
    xh8 = xs16.astype(e4)
    xl8 = (xs16 - xh8.astype(np.float32)).astype(e4)
    xh = tile_x(xh8)
    xl = tile_x(xl8)
    # wo: transpose, 64x scale, fp8 hi/lo, tile [p, mc, jt, m]
    JT, MC2 = 2 * SEQ // P, DIM // 256
    ws64 = 64.0 * np.asarray(wo, np.float32).T
    woh8 = ws64.astype(e4)
    wol8 = (ws64 - woh8.astype(np.float32)).astype(e4)

    def tile_wo(a):
        return np.ascontiguousarray(
            a.reshape(JT, P, MC2, 256).transpose(1, 2, 0, 3)
        )

    woh = tile_wo(woh8)
    wol = tile_wo(wol8)
    mul_mask = any(e[4] for row in plan for e in row if e[1] >= 0)
    mb_dt = bf16 if mul_mask else np.float32
    if n_uniq:
        mbs = np.ascontiguousarray(np.stack(blocks, axis=0)).astype(mb_dt)
    else:
        mbs = np.zeros((1, P, 512), mb_dt)

    in_maps = []
    for c in range(CORES):
        w_c = np.concatenate(
            [
                wq[c * NH * D : (c + 1) * NH * D],
                wk[c * D : (c + 1) * D],
                wv[c * D : (c + 1) * D],
            ],
            axis=0,
        ).astype(np.float32)  # (768, DIM)
        # w?[p, t, f] ~ w_c[f, 128t+p], scaled 64x and split hi/lo in fp8
        ws64 = (64.0 * w_c.T).reshape(DD, P, 768).transpose(1, 0, 2)
        wh8 = ws64.astype(e4)
        wl8 = (ws64 - wh8.astype(np.float32)).astype(e4)
        in_maps.append(
            {
                "xh": xh, "xl": xl,
                "wh": np.ascontiguousarray(wh8),
                "wl": np.ascontiguousarray(wl8),
                "cs": cs, "maskb": mbs, "woh": woh, "wol": wol,
            }
        )
    return in_maps


_BUILD_CACHE = {}


def kernel(
    x,
    cos_freq,
    sin_freq,
    positions,
    mask,
    wq,
    wk,
    wv,
    wo,
    _trace=False,
):
    import sys

    if "/opt/trn_rl_repo" not in sys.path:
        sys.path.insert(0, "/opt/trn_rl_repo")
    from concourse.bass_utils import run_bass_kernel_spmd

    x = np.asarray(x, np.float32)
    mask = np.asarray(mask, np.float32)
    wq = np.asarray(wq, np.float32)
    wk = np.asarray(wk, np.float32)
    wv = np.asarray(wv, np.float32)
    wo = np.asarray(wo, np.float32)
    SEQ, DIM = x.shape
    assert wq.shape[0] == CORES * NH * D and wk.shape[0] == CORES * D
    assert 2 * SEQ == wq.shape[0], "flatten structure requires H*D == 2*SEQ"

    plan, blocks = analyze_mask(mask, SEQ)
    n_uniq = len(blocks)
    key = (SEQ, DIM, tuple(tuple(r) for r in plan))
    if key not in _BUILD_CACHE:
        _BUILD_CACHE[key] = build_attention_nc(SEQ, DIM, plan, n_uniq)
    nc = _BUILD_CACHE[key]

    in_maps = stage_in_maps(
        x, cos_freq, sin_freq, wq, wk, wv, wo, plan, blocks
    )

    import time as _time

    _t0 = _time.time()
    res = run_bass_kernel_spmd(nc, in_maps, list(range(CORES)), trace=_trace)
    global LAST_EXEC_NS
    LAST_EXEC_NS = int((_time.time() - _t0) * 1e9)
    outp = np.concatenate(
        [res.results[c]["out"] for c in range(CORES)], axis=0
    ).astype(np.float32)
    if _trace:
        return outp, res
    return outp


# revision 77
# speedup vs baseline: 1.0035x; 1.0035x over previous
"""Trainium2 Bass kernel for nn_Attention (GQA + RoPE + sliding-window mask).

Sharding: tensor-parallel over heads across 8 cores. Each core gets 4 q heads
and exactly 1 kv head (32 q / 8 kv heads, GQA group = 4). The reference's
quirky output flatten ((H,S,D)->(H,D,S)->reshape(S, H*D)) makes the final
projection contract over (d-parity, sequence) instead of heads, so the final
output is row-sharded by head block: core c produces rows [256c, 256c+256) of
the (2048, 4096) result with NO collective at all.

Design notes (all on one NeuronCore, same program on all 8 = pure SPMD).
Phases are kept DENSE and separate: the CoreSim cost model resets the PE
p-state ramp on every idle gap (half clock for 3us after a gap), so a
fragmented PE stream loses more than interleaving gains.
  phase 1: QKV projections as 3-term fp8 DoubleRow matmuls
           (x_hi*w_hi + x_lo*w_hi + x_hi*w_lo at 0.5 cycles/row and 256-deep
           contraction per instruction = 3/4 the PE cost of bf16, and MORE
           accurate: hi+lo carries ~7 mantissa bits vs bf16's 8). Host
           stages fp8(16x) / fp8(64w) splits; 1/1024 is folded into the
           rope tables and the V-copy scale. RoPE on DVE, PE transposes
           into bf16 [d, s] layouts for Q^T / K^T; V kept [s, d].
  phase 2: TRANSPOSED scores S^T[k, q] = (K^T tile)^T @ Q^T (stationary K
           tile, moving Q) so the exp output P^T[k, q] feeds the PV matmul
           directly as the moving operand -- no P transpose at all. Softmax
           runs WITHOUT the max pass (logits here are bounded ~|20|, and
           exp(-1e9)=0 handles the mask), so exp reads PSUM scores directly.
           Head-PAIR blocks share one [P, 1024] score tile and ONE fused exp
           per key tile (halves ACT per-op overhead; ACT is the phase-2
           bottleneck). A pure 0/-inf mask is applied MULTIPLICATIVELY (0/1,
           bf16, DVE 2x) to P^T after the exp, off the matmul->exp critical
           path; PV is emitted one unit late so exp latency hides under the
           next unit's matmuls; per-block store segments are lagged into the
           next block so the exp stream never breaks. Denominators come from
           1-column ones-matmuls on the PE (cost ~zero: matmul cost is
           output-free-size only; the interp allows one live accumulation
           group per PSUM bank, so the 8 chains run sequentially),
           reciprocals on DVE, applied as ACT scale-copies when storing A
           (walrus rejects InstTensorScalarPtr, the DVE equivalent).
  phase 3: final projection as 3-term fp8 DoubleRow vs wo (host-split
           fp8(64*wo)); 16*A is split on-device into fp8 hi+residual. wo
           streams as 256-col chunks through a 6-deep ring; output DMAs
           issue from ACT because SP's queue is busy streaming wo.
           The first 6 row-tile-0 blocks are INJECTED between phase-2's
           second-half softmax blocks (row tile 0's A completes at the
           phase-2 midpoint): they fill phase 2's idle PE with dense DR
           work, evacuating via DVE+SP so the ACT exp stream is untouched,
           and their consumed wo ring slots re-stage on the idle DMA device.
           The 8th PSUM bank this needs comes from packing the denominator
           accumulators (bitcast f32 region) into the Ap-transpose bank.
"""

import numpy as np
from contextlib import ExitStack

P = 128
D = 128  # head dim
NH = 4   # q heads per core
CORES = 8
NEG_THRESH = -1e8


def build_attention_nc(SEQ, DIM, plan, n_uniq):
    """Build the per-core Bass program.

    plan: list over q-supers qs (SEQ//512 entries) of lists of
          (kt, uid, lead, w) over included 128-wide key tiles; uid == -1
          means no mask add, else index into the (transposed) maskb tensor,
          applied to cols [128*lead, 128*lead + w). lead = number of leading
          fully-masked 128-col spans (skipped in matmul/exp, zero-filled in
          P^T). kt absent = fully masked.
    """
    import concourse.bass as bass
    import concourse.bacc as bacc
    import concourse.mybir as mybir
    import concourse.tile as tile
    from concourse.masks import make_identity

    f32 = mybir.dt.float32
    bf16 = mybir.dt.bfloat16
    A_ = mybir.AluOpType
    AF = mybir.ActivationFunctionType

    ST = SEQ // P          # 16 s-tiles
    DD = DIM // P          # 32 contraction tiles
    QS = SEQ // 512        # 4 query supers
    JT = 2 * SEQ // P      # 32 j-tiles for final matmul
    MC = DIM // 512        # 8 output chunks
    ITILES = (NH * 64) // P  # 2 output row tiles
    EW = NH * D            # 512 q features per core
    assert NH == 4 and SEQ % 512 == 0 and DIM % 512 == 0

    nc = bacc.Bacc(trn_type="TRN2", debug=False, num_devices=CORES)

    e4 = mybir.dt.float8e4
    DR = mybir.MatmulPerfMode.DoubleRow
    # x^T tiled by dim-tile: x?[p, t, s] = fp8(16*x)[s, 128t+p], split into a
    # hi part and an fp8 residual (hi+lo carries ~7 mantissa bits, better
    # than bf16). The 16x/64x scales keep the residuals out of fp8-subnormal
    # territory; the combined 1024x is folded into the rope tables and the
    # V-copy scale. fp8 DoubleRow matmuls pair two 128-contraction tiles per
    # instruction at 0.5 cycles/row: the 3-term (hh + lh + hl) projection
    # runs at ~3/4 the PE cost of bf16. Layout contiguous in s so streamed
    # 512-token slices have 512B runs per partition (no DMA penalty).
    xh = nc.dram_tensor("xh", [P, DD, SEQ], e4, kind="ExternalInput").ap()
    xl = nc.dram_tensor("xl", [P, DD, SEQ], e4, kind="ExternalInput").ap()
    # packed projection weights [wq_c; wk_c; wv_c]: w?[p, t, f] ~ w[f, 128t+p]
    wh = nc.dram_tensor("wh", [P, DD, EW + 2 * D], e4, kind="ExternalInput").ap()
    wl = nc.dram_tensor("wl", [P, DD, EW + 2 * D], e4, kind="ExternalInput").ap()
    # rope tables [cos_rep | sin_rep] with sqrt(scale) folded in
    cs = nc.dram_tensor("cs", [SEQ, EW], bf16, kind="ExternalInput").ap()
    # transposed partial mask blocks [k 128, q <=512], left-aligned;
    # bf16 0/1 factors when the mask is multiplicative, else f32 additive
    mul_mask = any(
        e[4] for row in plan for e in row if e[1] >= 0
    )
    mb = nc.dram_tensor(
        "maskb", [max(n_uniq, 1), P, 512], bf16 if mul_mask else f32,
        kind="ExternalInput",
    ).ap()
    # wo transposed, 64x-scaled, fp8 hi/lo split, pre-tiled [p, mc, jt, m] so
    # each 256-col chunk is one DMA with 8KB-contiguous per-partition runs
    MC2 = DIM // 256
    woh = nc.dram_tensor(
        "woh", [P, MC2, JT, 256], e4, kind="ExternalInput"
    ).ap()
    wol = nc.dram_tensor(
        "wol", [P, MC2, JT, 256], e4, kind="ExternalInput"
    ).ap()
    out = nc.dram_tensor("out", [NH * 64, DIM], bf16, kind="ExternalOutput").ap()

    with tile.TileContext(nc) as tc, ExitStack() as ctx:
        const = ctx.enter_context(tc.tile_pool(name="const", bufs=1))
        idP = const.tile([P, P], bf16)
        make_identity(nc, idP)
        # 1/16 folds the fp8 A-staging scale into the softmax denominators:
        # dpt = sum(P)/16 -> recip = 16/sum -> Aall holds 16*A (fp8-friendly)
        ones = const.tile([P, 1], bf16)
        nc.vector.memset(ones, 1.0 / 16.0)
        # 1/1024 row for DVE-side output descaling (plain TT is walrus-safe)
        sc1024 = const.tile([P, 256], f32)
        nc.vector.memset(sc1024, 1.0 / 1024.0)

        pers = ctx.enter_context(tc.tile_pool(name="pers", bufs=1))
        QT = pers.tile([P, NH, SEQ], bf16)   # [d, h, s]
        KT = pers.tile([P, SEQ], bf16)       # [d, s]
        Vt = pers.tile([P, ST, D], bf16)     # [s(part), stile, d]
        if n_uniq > 0:
            mbt = pers.tile([P, n_uniq, 512], bf16 if mul_mask else f32)
        Aall = [
            pers.tile([P, 2 * ST * D], bf16, name=f"Aall{i}")
            for i in range(ITILES)
        ]
        A8h = [
            pers.tile([P, 2 * ST * D], e4, name=f"A8h{i}")
            for i in range(ITILES)
        ]
        A8l = [
            pers.tile([P, 2 * ST * D], e4, name=f"A8l{i}")
            for i in range(ITILES)
        ]
        stat = ctx.enter_context(tc.tile_pool(name="stat", bufs=16))

        # ---------------- phase 1 ----------------
        with (
            tc.tile_pool(name="wpool", bufs=1) as wpool,
            tc.tile_pool(name="xpool", bufs=2) as xpool,
            tc.tile_pool(name="cspool", bufs=4) as cspool,
            tc.tile_pool(name="rpool", bufs=2) as rpool,
            tc.tile_pool(name="qps", bufs=2, space="PSUM") as qps,
            tc.tile_pool(name="kvps", bufs=2, space="PSUM") as kvps,
            tc.tile_pool(name="tps", bufs=2, space="PSUM") as tps,
        ):
            # startup-latency-aware DMA order: the first (hi*hi) matmuls need
            # only wh t-tiles 0..3 and the first half of xh s-group 0; the
            # hi*lo and lo*hi terms follow, so wl/xl stream behind
            wht = wpool.tile([P, DD, EW + 2 * D], e4, name="wht")
            wlt = wpool.tile([P, DD, EW + 2 * D], e4, name="wlt")
            xs0h = xpool.tile([P, DD, 512], e4, tag="xsh", name="xs0h")
            xs0l = xpool.tile([P, DD, 512], e4, tag="xsl", name="xs0l")
            # supply order tracks the term order hh -> hl -> lh of s-group 0,
            # in fine-grained chunks so the first matmuls start after ~2 DMAs
            cst0 = [cspool.tile([P, EW], bf16, tag="cs", name=f"cst{l}")
                    for l in range(4)]
            for g in range(8):
                nc.sync.dma_start(
                    out=wht[:, 4 * g : 4 * g + 4, :],
                    in_=wh[:, 4 * g : 4 * g + 4, :],
                )
                nc.sync.dma_start(
                    out=xs0h[:, 4 * g : 4 * g + 4, :],
                    in_=xh[:, 4 * g : 4 * g + 4, 0:512],
                )
                if g == 1:
                    for l in range(2):
                        nc.sync.dma_start(
                            out=cst0[l], in_=cs[l * P : (l + 1) * P, :]
                        )
            for g in range(8):
                nc.sync.dma_start(
                    out=wlt[:, 4 * g : 4 * g + 4, :],
                    in_=wl[:, 4 * g : 4 * g + 4, :],
                )
                nc.sync.dma_start(
                    out=xs0l[:, 4 * g : 4 * g + 4, :],
                    in_=xl[:, 4 * g : 4 * g + 4, 0:512],
                )
                if g == 1:
                    for l in range(2, 4):
                        nc.sync.dma_start(
                            out=cst0[l], in_=cs[l * P : (l + 1) * P, :]
                        )
            def finish_stile(st, cst, Qp, KVp):
                # rope via strided even/odd halves; final add/sub writes
                # bf16 so the PE transposes below run at 1 cycle/row
                HF = EW // 2
                rq = rpool.tile([P, EW], bf16, tag="rq")
                rk = rpool.tile([P, D], bf16, tag="rk")
                t1 = rpool.tile([P, HF], f32, tag="t1")
                t2 = rpool.tile([P, HF], f32, tag="t2")
                q_ev, q_od = Qp[:, 0:EW:2], Qp[:, 1:EW:2]
                cosr, sinr = cst[:, 0:HF], cst[:, HF : 2 * HF]
                nc.vector.tensor_tensor(out=t1, in0=q_ev, in1=cosr, op=A_.mult)
                nc.vector.tensor_tensor(out=t2, in0=q_od, in1=sinr, op=A_.mult)
                nc.vector.tensor_tensor(
                    out=rq[:, 0:EW:2], in0=t1, in1=t2, op=A_.subtract
                )
                nc.vector.tensor_tensor(out=t1, in0=q_ev, in1=sinr, op=A_.mult)
                nc.vector.tensor_tensor(out=t2, in0=q_od, in1=cosr, op=A_.mult)
                nc.vector.tensor_tensor(
                    out=rq[:, 1:EW:2], in0=t1, in1=t2, op=A_.add
                )

                k_ev, k_od = KVp[:, 0:D:2], KVp[:, 1:D:2]
                cosk, sink = cst[:, 0 : D // 2], cst[:, HF : HF + D // 2]
                t1k, t2k = t1[:, 0 : D // 2], t2[:, 0 : D // 2]
                nc.vector.tensor_tensor(out=t1k, in0=k_ev, in1=cosk, op=A_.mult)
                nc.vector.tensor_tensor(out=t2k, in0=k_od, in1=sink, op=A_.mult)
                nc.vector.tensor_tensor(
                    out=rk[:, 0:D:2], in0=t1k, in1=t2k, op=A_.subtract
                )
                nc.vector.tensor_tensor(out=t1k, in0=k_ev, in1=sink, op=A_.mult)
                nc.vector.tensor_tensor(out=t2k, in0=k_od, in1=cosk, op=A_.mult)
                nc.vector.tensor_tensor(
                    out=rk[:, 1:D:2], in0=t1k, in1=t2k, op=A_.add
                )

                # V -> bf16 [s, d] (ACT copy, cast; 1/1024 undoes the
                # fp8 staging scales 16x on x and 64x on w)
                nc.scalar.activation(
                    out=Vt[:, st, :], in_=KVp[:, D : 2 * D], func=AF.Copy,
                    scale=1.0 / 1024.0,
                )

                # transpose rq (per head) and rk into bf16 [d, s] layouts
                Tt = tps.tile([P, EW + D], bf16, tag="T")
                for h in range(NH):
                    nc.tensor.transpose(
                        Tt[:, h * P : (h + 1) * P], rq[:, h * P : (h + 1) * P],
                        idP,
                    )
                nc.tensor.transpose(Tt[:, EW : EW + D], rk, idP)
                nc.scalar.activation(
                    out=QT[:, :, st * P : (st + 1) * P],
                    in_=Tt[:, 0:EW].rearrange("p (h s) -> p h s", h=NH),
                    func=AF.Copy,
                )
                nc.scalar.activation(
                    out=KT[:, st * P : (st + 1) * P],
                    in_=Tt[:, EW : EW + D],
                    func=AF.Copy,
                )

            SG = ST // 4  # stream x in 4 groups of 4 s-tiles
            for sg in range(SG):
                if sg == 0:
                    xsh, xsl = xs0h, xs0l
                    csts = cst0
                else:
                    xsh = xpool.tile([P, DD, 512], e4, tag="xsh")
                    nc.sync.dma_start(
                        out=xsh, in_=xh[:, :, sg * 512 : (sg + 1) * 512]
                    )
                    xsl = xpool.tile([P, DD, 512], e4, tag="xsl")
                    nc.sync.dma_start(
                        out=xsl, in_=xl[:, :, sg * 512 : (sg + 1) * 512]
                    )
                    csts = []
                    for l in range(4):
                        st = 4 * sg + l
                        c = cspool.tile([P, EW], bf16, tag="cs")
                        nc.sync.dma_start(
                            out=c, in_=cs[st * P : (st + 1) * P, :]
                        )
                        csts.append(c)
                TP = DD // 2
                terms = [(xsh, wht), (xsh, wlt), (xsl, wht)]

                def emit_mm(ls, tiles):
                    # emit the 3-term DR chains for s-tiles `ls`, interleaved
                    # tp-major so demand tracks the chunked DMA supply order;
                    # each s-tile's rope/store is emitted as soon as ITS
                    # chain closes so the psum bufs recycle early
                    for term, (xst, wt) in enumerate(terms):
                        for tp in range(TP):
                            for i, l in enumerate(ls):
                                Qp, KVp = tiles[i]
                                lhsT = xst[:, 2 * tp : 2 * tp + 2,
                                           l * P : (l + 1) * P]
                                first = term == 0 and tp == 0
                                last = term == 2 and tp == TP - 1
                                nc.tensor.matmul(
                                    Qp, lhsT,
                                    wt[:, 2 * tp : 2 * tp + 2, 0:EW],
                                    start=first, stop=last, perf_mode=DR,
                                )
                                nc.tensor.matmul(
                                    KVp, lhsT,
                                    wt[:, 2 * tp : 2 * tp + 2,
                                       EW : EW + 2 * D],
                                    start=first, stop=last, perf_mode=DR,
                                )
                            if term == 2 and tp == TP - 1:
                                for i, l in enumerate(ls):
                                    finish_stile(4 * sg + l, csts[l],
                                                 *tiles[i])

                for ls in [(0,), (1,), (2,), (3,)]:
                    tiles = [
                        (qps.tile([P, EW], f32, tag="Qp", name=f"Qp{l}"),
                         kvps.tile([P, 2 * D], f32, tag="KVp", name=f"KVp{l}"))
                        for l in ls
                    ]
                    emit_mm(ls, tiles)
        # ---------------- phase 2: head-pair blocks ----------------
        # per unit (key tile kt): scores for BOTH heads of the pair into one
        # [P, 1024] psum tile, ONE fused exp over both halves, deferred PV.
        # Each block's store segment (recips / A store) is emitted LAGGED,
        # after the next block's first unit, so the ACT exp stream never
        # breaks at block boundaries.
        wopool = ctx.enter_context(tc.tile_pool(name="wopool", bufs=6))
        osb = ctx.enter_context(tc.tile_pool(name="osb", bufs=2))
        with (
            tc.tile_pool(name="ptpool", bufs=20) as ptpool,
            tc.tile_pool(name="atsb", bufs=4) as atsb,
            tc.tile_pool(name="sps", bufs=2, space="PSUM") as sps,
            tc.tile_pool(name="atps", bufs=2, space="PSUM") as atps,
            # one [P, 1024] bf16 bank carries BOTH the Ap transposes (cols
            # 0:512) and, bitcast to f32, the denominator accumulators (cols
            # 512:528) -- PSUM slots are bank-granular, and packing them
            # frees the 8th bank for the ops_inj pool below
            tc.tile_pool(name="aps", bufs=1, space="PSUM") as aps,
            tc.tile_pool(name="ops_inj", bufs=1, space="PSUM") as ops_inj,
        ):
            # wo prefetch (SP queue position: right after phase-1 DMAs; the
            # DMA device is idle during phase 2). 256-wide chunks, hi+lo fp8.
            wots = {}

            def stage_wo(mc):
                th = wopool.tile([P, JT, 256], e4, tag="woh", name=f"woh{mc}")
                tl = wopool.tile([P, JT, 256], e4, tag="wol", name=f"wol{mc}")
                nc.sync.dma_start(out=th, in_=woh[:, mc, :, :])
                nc.sync.dma_start(out=tl, in_=wol[:, mc, :, :])
                wots[mc] = (th, tl)

            if n_uniq > 0:
                nc.sync.dma_start(out=mbt, in_=mb.rearrange("u p m -> p u m"))
            for mc in range(6):
                stage_wo(mc)

            def pair_block_gen(hp, qs):
                """hp in {0,1}: heads (2hp, 2hp+1). Yields per unit; the
                final segment (after last yield) is the block tail."""
                row = plan[qs]
                h0 = 2 * hp
                if not row:
                    for qi in range(4):
                        i = 4 * qs + qi
                        nc.vector.memset(
                            Aall[hp][:, i * 2 * P : (i + 1) * 2 * P], 0.0
                        )
                    yield
                    return
                Ats = [atps.tile([P, 512], f32, tag="At", name=f"At{hl}")
                       for hl in range(2)]
                mt = aps.tile([P, 1024], bf16, tag="Ap")
                Ap = mt[:, 0:512]
                dpt = mt[:, 512:528].bitcast(f32)  # [P, 8] f32 region
                pts = []
                pend = None  # deferred PV (one unit of lag hides exp latency)
                for n, (kt, uid, lead, w, mul) in enumerate(row):
                    if pend is not None:
                        pkt, pPT, pn = pend
                        for hl in range(2):
                            nc.tensor.matmul(
                                Ats[hl], Vt[:, pkt, :],
                                pPT[:, hl * 512 : (hl + 1) * 512],
                                start=(pn == 0), stop=False,
                            )
                    off = lead * P
                    S = sps.tile([P, 1024], f32, tag="S")
                    for hl in range(2):
                        nc.tensor.matmul(
                            S[:, hl * 512 + off : (hl + 1) * 512],
                            KT[:, kt * P : (kt + 1) * P],
                            QT[:, h0 + hl, qs * 512 + off : (qs + 1) * 512],
                            start=True, stop=True,
                        )
                        if uid >= 0 and not mul:
                            # additive mask must run pre-exp (ACT waits on it)
                            sl = S[:, hl * 512 + off : hl * 512 + off + w]
                            nc.vector.tensor_add(sl, sl, mbt[:, uid, 0:w])
                    PT = ptpool.tile([P, 1024], bf16, tag="PT")
                    sv = S.rearrange("p (hl q) -> p hl q", hl=2)[:, :, off:512]
                    pv = PT.rearrange("p (hl q) -> p hl q", hl=2)[:, :, off:512]
                    nc.scalar.activation(out=pv, in_=sv, func=AF.Exp)
                    if uid >= 0 and mul:
                        # 0/1 mask applied to P post-exp (all-bf16 -> DVE 2x
                        # mode; the deferred PV gives it a unit of slack)
                        for hl in range(2):
                            sl = PT[:, hl * 512 + off : hl * 512 + off + w]
                            nc.vector.tensor_tensor(
                                out=sl, in0=sl, in1=mbt[:, uid, 0:w],
                                op=A_.mult,
                            )
                    if off:
                        for hl in range(2):
                            nc.vector.memset(
                                PT[:, hl * 512 : hl * 512 + off], 0.0
                            )
                    pts.append(PT)
                    pend = (kt, PT, n)
                    yield
                pkt, pPT, pn = pend
                for hl in range(2):
                    nc.tensor.matmul(
                        Ats[hl], Vt[:, pkt, :],
                        pPT[:, hl * 512 : (hl + 1) * 512],
                        start=(pn == 0), stop=True,
                    )
                # denominators via 1-wide ones-matmuls (the interp models one
                # live accumulation group per PSUM bank, so the 8 chains run
                # one after another). Emitted in-block; the store segment
                # below is lagged into the next block.
                for hl in range(2):
                    for qi in range(4):
                        col = hl * 4 + qi
                        for n, PT in enumerate(pts):
                            nc.tensor.matmul(
                                dpt[:, col : col + 1],
                                PT[:, hl * 512 + qi * P : hl * 512 + (qi + 1) * P],
                                ones,
                                start=(n == 0), stop=(n == len(pts) - 1),
                                skip_group_check=True,
                            )
                yield  # --- lag point: store segment pulled later ---
                for hl in range(2):
                    recips = []
                    for qi in range(4):
                        col = hl * 4 + qi
                        r = stat.tile([P, 1], f32, tag="recip")
                        nc.vector.reciprocal(r, dpt[:, col : col + 1])
                        recips.append(r)
                    Atsb = atsb.tile([P, 512], bf16, tag="Atsb")
                    nc.vector.tensor_copy(out=Atsb, in_=Ats[hl])
                    for qi in range(4):
                        nc.tensor.transpose(
                            Ap[:, qi * P : (qi + 1) * P],
                            Atsb[:, qi * P : (qi + 1) * P],
                            idP,
                        )
                    # Aall layout: [sp, (t*2 + dd)*128 + hb*64 + p] so the
                    # final matmul's stationary slices are contiguous
                    Ah = Aall[hp]
                    for qi in range(4):
                        i = 4 * qs + qi
                        dview = Ah[:, i * 2 * P : (i + 1) * 2 * P].rearrange(
                            "a (dd j) -> a dd j", dd=2
                        )[:, :, hl * 64 : hl * 64 + 64].rearrange(
                            "a dd p -> a p dd"
                        )
                        src = Ap[:, qi * P : (qi + 1) * P].rearrange(
                            "a (p two) -> a p two", two=2
                        )
                        # per-partition scale-copy on ACT (walrus rejects the
                        # DVE InstTensorScalarPtr equivalent)
                        nc.scalar.activation(
                            out=dview, in_=src, func=AF.Copy,
                            scale=recips[qi],
                        )

            def p3_gen(it, mc, opool, on_act):
                """Yield after each of the 3 DR terms so injection can feed
                the PE in ~0.9us segments without starving the ACT exps."""
                woth, wotl = wots[mc]
                O = opool.tile([P, 256], f32, tag="O")
                # 3-term fp8 DoubleRow: (Ah+Al)@(wh+wl) minus the Al@wl
                # term; each DR instruction contracts a PAIR of j-tiles
                terms = [(A8h[it], woth), (A8h[it], wotl), (A8l[it], woth)]
                for term, (Av, wv) in enumerate(terms):
                    Av4 = Av.rearrange("p (t d j) -> p t d j", d=2, j=P)
                    for ddj in range(2):
                        for u in range(ST // 2):
                            first = term == 0 and ddj == 0 and u == 0
                            last = (term == 2 and ddj == 1
                                    and u == ST // 2 - 1)
                            nc.tensor.matmul(
                                O,
                                Av4[:, 2 * u : 2 * u + 2, ddj, :],
                                wv[:, ddj * ST + 2 * u
                                   : ddj * ST + 2 * u + 2, :],
                                start=first, stop=last, perf_mode=DR,
                            )
                    if term < 2:
                        yield
                Ot = osb.tile([P, 256], bf16, tag="Ot")
                # 1/1024 undoes the 16x (A) and 64x (wo) fp8 scales; during
                # phase 2 the evacuation runs on DVE (ACT is the softmax
                # bottleneck), in the tail on ACT (SP busy streaming wo)
                if on_act:
                    nc.scalar.activation(
                        out=Ot, in_=O, func=AF.Copy, scale=1.0 / 1024.0
                    )
                    nc.scalar.dma_start(
                        out=out[it * P : (it + 1) * P,
                                mc * 256 : (mc + 1) * 256],
                        in_=Ot,
                    )
                else:
                    nc.vector.tensor_tensor(
                        out=Ot, in0=O, in1=sc1024, op=A_.mult
                    )
                    nc.sync.dma_start(
                        out=out[it * P : (it + 1) * P,
                                mc * 256 : (mc + 1) * 256],
                        in_=Ot,
                    )

            def p3_block(it, mc, opool, on_act):
                for _ in p3_gen(it, mc, opool, on_act):
                    pass

            def split_a8(it, chunks=4, on_act=True):
                # split 16*A (bf16) into fp8 hi + residual
                for ch in range(chunks):
                    sl = slice(ch * (2 * ST * D // chunks),
                               (ch + 1) * (2 * ST * D // chunks))
                    if on_act:
                        nc.scalar.activation(
                            out=A8h[it][:, sl], in_=Aall[it][:, sl],
                            func=AF.Copy,
                        )
                    else:
                        nc.vector.tensor_copy(
                            out=A8h[it][:, sl], in_=Aall[it][:, sl]
                        )
                    nc.vector.tensor_tensor(
                        out=A8l[it][:, sl], in0=Aall[it][:, sl],
                        in1=A8h[it][:, sl], op=A_.subtract,
                    )

            # drive blocks hp-major; pull the previous block's store segment
            # right after the current block's FIRST unit (before the current
            # block's first PV touches the recycled At banks). Once hp0's A
            # is complete, early it0 output-projection blocks are injected
            # one DR TERM per softmax unit: the PE fills its idle time with
            # dense work while the ACT exp stream stays fed, all injected
            # evacuation off the ACT critical path. Consumed wo ring slots
            # are immediately re-staged on the idle phase-2 DMA device.
            N_INJ = 6
            inj = {"gen": None, "mc": 0, "stage": [0, 1, 2, 3, 4, 5]}

            def pump_inj():
                if inj["gen"] is None:
                    if inj["mc"] >= N_INJ:
                        return
                    inj["gen"] = p3_gen(0, inj["mc"], ops_inj, on_act=False)
                try:
                    next(inj["gen"])
                except StopIteration:
                    inj["gen"] = None
                    inj["mc"] += 1
                    if inj["stage"]:
                        stage_wo(inj["stage"].pop(0))

            pending = None
            for hp in range(2):
                for qs in range(QS):
                    g = pair_block_gen(hp, qs)
                    L = len(plan[qs])
                    nunits = max(1, L)
                    for n in range(nunits + (1 if L else 0)):
                        next(g)  # units, then the denominator segment
                        if n == 0 and pending is not None:
                            try:
                                next(pending)
                            except StopIteration:
                                pass
                            pending = None
                            if hp == 1 and qs == 0:
                                # hp0's last store just emitted: A of row
                                # tile 0 is complete -> split it on DVE
                                split_a8(0, on_act=False)
                    if pending is not None:  # empty-row block: flush now
                        try:
                            next(pending)
                        except StopIteration:
                            pass
                    pending = g if L else None
                    if hp == 1 and qs >= 1:
                        for _ in range(6):  # two whole blocks, term-paced
                            pump_inj()
            if pending is not None:
                try:
                    next(pending)
                except StopIteration:
                    pass
            # it1's A is now complete: split it on DVE under the remaining
            # injected blocks so the tail starts unblocked
            split_a8(1, on_act=False)
            while inj["mc"] < N_INJ:
                pump_inj()

        # ---------------- phase 3 tail ----------------
        # chunks 6..9 were consumed by injected it0 blocks and not yet
        # re-staged; chunks 10..15 get their first staging inside the loop
        for mc in range(6, N_INJ):
            stage_wo(mc)
        with tc.tile_pool(name="ops", bufs=4, space="PSUM") as ops:
            for mc in range(MC2):
                if N_INJ <= mc + 6 < MC2:
                    stage_wo(mc + 6)
                for it in range(ITILES):
                    if it == 0 and mc < N_INJ:
                        continue  # done during phase 2
                    p3_block(it, mc, ops, on_act=True)

    nc.compile()
    return nc


def analyze_mask(mask, SEQ):
    """Classify transposed 128-key x 512-query mask blocks per (kt, qs):
    skip / free / masked(dedup uid). Partial blocks store only the masked
    window: lead = # of leading fully-masked 128-col spans, w = width of the
    remaining span that contains any masked element. Blocks stored TRANSPOSED
    [k, q], left-aligned into a [128, 512] buffer."""
    QS = SEQ // 512
    KTOT = SEQ // P
    uniq = {}
    blocks = []
    plan = []
    # every query row needs at least one allowed key (no-max softmax would
    # otherwise divide by zero; the reference's uniform-distribution quirk
    # for fully-masked rows is not representable in this fast path)
    assert (mask > NEG_THRESH).any(axis=1).all(), "fully masked query row"
    # a pure 0/-inf mask can be applied MULTIPLICATIVELY (0/1) to P after the
    # exp, off the matmul->exp critical path; finite-valued masks must stay
    # additive pre-exp
    mul = bool(((mask <= NEG_THRESH) | (mask == 0.0)).all())
    for qs in range(QS):
        row = []
        for kt in range(KTOT):
            blk = mask[qs * 512 : (qs + 1) * 512, kt * P : (kt + 1) * P]
            if (blk <= NEG_THRESH).all():
                continue
            if not blk.any():
                row.append((kt, -1, 0, 0, mul))
                continue
            bT = np.ascontiguousarray(blk.T)  # [128 k, 512 q]
            col_all = (bT <= NEG_THRESH).all(axis=0)
            col_any = bT.any(axis=0)
            lead = 0
            while lead < 3 and col_all[lead * P : (lead + 1) * P].all():
                lead += 1
            last_any = int(np.nonzero(col_any)[0].max())
            w = (last_any // P + 1) * P - lead * P
            sl = bT[:, lead * P : lead * P + w]
            key = (w, sl.tobytes())
            if key not in uniq:
                uniq[key] = len(blocks)
                buf = np.zeros((P, 512), np.float32)
                buf[:, 0:w] = (sl > NEG_THRESH) if mul else sl
                blocks.append(buf)
            row.append((kt, uniq[key], lead, w, mul))
        plan.append(row)
    return plan, blocks


def make_rope_tables(cos_freq, sin_freq, SEQ, scale_quarter):
    """[cos_rep (SEQ, NH*64) | sin_rep (SEQ, NH*64)], sqrt(scale) folded in."""
    cos_t = np.tile(np.asarray(cos_freq, np.float32) * scale_quarter, (1, NH))
    sin_t = np.tile(np.asarray(sin_freq, np.float32) * scale_quarter, (1, NH))
    import ml_dtypes
    return np.ascontiguousarray(
        np.concatenate([cos_t, sin_t], axis=1).astype(ml_dtypes.bfloat16)
    )


def stage_in_maps(x, cos_freq, sin_freq, wq, wk, wv, wo, plan, blocks):
    """Host-side input staging (shared by kernel() and test harnesses)."""
    import ml_dtypes

    bf16 = ml_dtypes.bfloat16
    e4 = ml_dtypes.float8_e4m3
    SEQ, DIM = x.shape
    DD = DIM // P
    n_uniq = len(blocks)
    # rope tables fold sqrt(scale) per side AND 1/1024 (fp8 staging scales)
    scale_quarter = np.float32(D ** -0.25) / np.float32(1024.0)
    cs = make_rope_tables(cos_freq, sin_freq, SEQ, scale_quarter)

    def tile_x(a):
        # [p, t, s] = a[s, 128t+p]
        return np.ascontiguousarray(a.reshape(SEQ, DD, P).transpose(2, 1, 0))

    xs16 = 16.0 * np.asarray(x, np.float32)
    xh8 = xs16.astype(e4)
    xl8 = (xs16 - xh8.astype(np.float32)).astype(e4)
    xh = tile_x(xh8)
    xl = tile_x(xl8)
    # wo: transpose, 64x scale, fp8 hi/lo, tile [p, mc, jt, m]
    JT, MC2 = 2 * SEQ // P, DIM // 256
    ws64 = 64.0 * np.asarray(wo, np.float32).T
    woh8 = ws64.astype(e4)
    wol8 = (ws64 - woh8.astype(np.float32)).astype(e4)

    def tile_wo(a):
        return np.ascontiguousarray(
            a.reshape(JT, P, MC2, 256).transpose(1, 2, 0, 3)
        )

    woh = tile_wo(woh8)
    wol = tile_wo(wol8)
    mul_mask = any(e[4] for row in plan for e in row if e[1] >= 0)
    mb_dt = bf16 if mul_mask else np.float32
    if n_uniq:
        mbs = np.ascontiguousarray(np.stack(blocks, axis=0)).astype(mb_dt)
    else:
        mbs = np.zeros((1, P, 512), mb_dt)

    in_maps = []
    for c in range(CORES):
        w_c = np.concatenate(
            [
                wq[c * NH * D : (c + 1) * NH * D],
                wk[c * D : (c + 1) * D],
                wv[c * D : (c + 1) * D],
            ],
            axis=0,
        ).astype(np.float32)  # (768, DIM)
        # w?[p, t, f] ~ w_c[f, 128t+p], scaled 64x and split hi/lo in fp8
        ws64 = (64.0 * w_c.T).reshape(DD, P, 768).transpose(1, 0, 2)
        wh8 = ws64.astype(e4)
        wl8 = (ws64 - wh8.astype(np.float32)).astype(e4)
        in_maps.append(
            {
                "xh": xh, "xl": xl,
                "wh": np.ascontiguousarray(wh8),
                "wl": np.ascontiguousarray(wl8),
                "cs": cs, "maskb": mbs, "woh": woh, "wol": wol,
            }
        )
    return in_maps


_BUILD_CACHE = {}


def kernel(
    x,
    cos_freq,
    sin_freq,
    positions,
    mask,
    wq,
    wk,
    wv,
    wo,
    _trace=False,
):
    import sys

    if "/opt/trn_rl_repo" not in sys.path:
        sys.path.insert(0, "/opt/trn_rl_repo")
    from concourse.bass_utils import run_bass_kernel_spmd

    x = np.asarray(x, np.float32)
    mask = np.asarray(mask, np.float32)
    wq = np.asarray(wq, np.float32)
    wk = np.asarray(wk, np.float32)
    wv = np.asarray(wv, np.float32)
    wo = np.asarray(wo, np.float32)
    SEQ, DIM = x.shape
    assert wq.shape[0] == CORES * NH * D and wk.shape[0] == CORES * D
    assert 2 * SEQ == wq.shape[0], "flatten structure requires H*D == 2*SEQ"

    plan, blocks = analyze_mask(mask, SEQ)
    n_uniq = len(blocks)
    key = (SEQ, DIM, tuple(tuple(r) for r in plan))
    if key not in _BUILD_CACHE:
        _BUILD_CACHE[key] = build_attention_nc(SEQ, DIM, plan, n_uniq)
    nc = _BUILD_CACHE[key]

    in_maps = stage_in_maps(
        x, cos_freq, sin_freq, wq, wk, wv, wo, plan, blocks
    )

    import time as _time

    _t0 = _time.time()
    res = run_bass_kernel_spmd(nc, in_maps, list(range(CORES)), trace=_trace)
    global LAST_EXEC_NS
    LAST_EXEC_NS = int((_time.time() - _t0) * 1e9)
    outp = np.concatenate(
        [res.results[c]["out"] for c in range(CORES)], axis=0
    ).astype(np.float32)
    if _trace:
        return outp, res
    return outp


# revision 84
# speedup vs baseline: 1.0105x; 1.0070x over previous
"""Trainium2 Bass kernel for nn_Attention (GQA + RoPE + sliding-window mask).

Sharding: tensor-parallel over heads across 8 cores. Each core gets 4 q heads
and exactly 1 kv head (32 q / 8 kv heads, GQA group = 4). The reference's
quirky output flatten ((H,S,D)->(H,D,S)->reshape(S, H*D)) makes the final
projection contract over (d-parity, sequence) instead of heads, so the final
output is row-sharded by head block: core c produces rows [256c, 256c+256) of
the (2048, 4096) result with NO collective at all.

Design notes (all on one NeuronCore, same program on all 8 = pure SPMD).
Phases are kept DENSE and separate: the CoreSim cost model resets the PE
p-state ramp on every idle gap (half clock for 3us after a gap), so a
fragmented PE stream loses more than interleaving gains.
  phase 1: QKV projections as 3-term fp8 DoubleRow matmuls
           (x_hi*w_hi + x_lo*w_hi + x_hi*w_lo at 0.5 cycles/row and 256-deep
           contraction per instruction = 3/4 the PE cost of bf16, and MORE
           accurate: hi+lo carries ~7 mantissa bits vs bf16's 8). Host
           stages fp8(16x) / fp8(64w) splits; 1/1024 is folded into the
           rope tables and the V-copy scale. RoPE on DVE, PE transposes
           into bf16 [d, s] layouts for Q^T / K^T; V kept [s, d].
  phase 2: TRANSPOSED scores S^T[k, q] = (K^T tile)^T @ Q^T (stationary K
           tile, moving Q) so the exp output P^T[k, q] feeds the PV matmul
           directly as the moving operand -- no P transpose at all. Softmax
           runs WITHOUT the max pass (logits here are bounded ~|20|, and
           exp(-1e9)=0 handles the mask), so exp reads PSUM scores directly.
           Head-PAIR blocks share one [P, 1024] score tile and ONE fused exp
           per key tile (halves ACT per-op overhead; ACT is the phase-2
           bottleneck). A pure 0/-inf mask is applied MULTIPLICATIVELY (0/1,
           bf16, DVE 2x) to P^T after the exp, off the matmul->exp critical
           path; PV is emitted one unit late so exp latency hides under the
           next unit's matmuls; per-block store segments are lagged into the
           next block so the exp stream never breaks. Denominators come from
           1-column ones-matmuls on the PE (cost ~zero: matmul cost is
           output-free-size only; the interp allows one live accumulation
           group per PSUM bank, so the 8 chains run sequentially),
           reciprocals on DVE, applied as ACT scale-copies when storing A
           (walrus rejects InstTensorScalarPtr, the DVE equivalent).
  phase 3: final projection as 3-term fp8 DoubleRow vs wo (host-split
           fp8(64*wo)); 16*A is split on-device into fp8 hi+residual. wo
           streams as 256-col chunks through a 6-deep ring; output DMAs
           issue from ACT because SP's queue is busy streaming wo.
           The first 6 row-tile-0 blocks are INJECTED between phase-2's
           second-half softmax blocks (row tile 0's A completes at the
           phase-2 midpoint): they fill phase 2's idle PE with dense DR
           work, evacuating via DVE+SP so the ACT exp stream is untouched,
           and their consumed wo ring slots re-stage on the idle DMA device.
           The 8th PSUM bank this needs comes from packing the denominator
           accumulators (bitcast f32 region) into the Ap-transpose bank.
"""

import numpy as np
from contextlib import ExitStack

P = 128
D = 128  # head dim
NH = 4   # q heads per core
CORES = 8
NEG_THRESH = -1e8


def build_attention_nc(SEQ, DIM, plan, n_uniq):
    """Build the per-core Bass program.

    plan: list over q-supers qs (SEQ//512 entries) of lists of
          (kt, uid, lead, w) over included 128-wide key tiles; uid == -1
          means no mask add, else index into the (transposed) maskb tensor,
          applied to cols [128*lead, 128*lead + w). lead = number of leading
          fully-masked 128-col spans (skipped in matmul/exp, zero-filled in
          P^T). kt absent = fully masked.
    """
    import concourse.bass as bass
    import concourse.bacc as bacc
    import concourse.mybir as mybir
    import concourse.tile as tile
    from concourse.masks import make_identity

    f32 = mybir.dt.float32
    bf16 = mybir.dt.bfloat16
    A_ = mybir.AluOpType
    AF = mybir.ActivationFunctionType

    ST = SEQ // P          # 16 s-tiles
    DD = DIM // P          # 32 contraction tiles
    QS = SEQ // 512        # 4 query supers
    JT = 2 * SEQ // P      # 32 j-tiles for final matmul
    MC = DIM // 512        # 8 output chunks
    ITILES = (NH * 64) // P  # 2 output row tiles
    EW = NH * D            # 512 q features per core
    assert NH == 4 and SEQ % 512 == 0 and DIM % 512 == 0

    nc = bacc.Bacc(trn_type="TRN2", debug=False, num_devices=CORES)

    e4 = mybir.dt.float8e4
    DR = mybir.MatmulPerfMode.DoubleRow
    # x^T tiled by dim-tile: x?[p, t, s] = fp8(16*x)[s, 128t+p], split into a
    # hi part and an fp8 residual (hi+lo carries ~7 mantissa bits, better
    # than bf16). The 16x/64x scales keep the residuals out of fp8-subnormal
    # territory; the combined 1024x is folded into the rope tables and the
    # V-copy scale. fp8 DoubleRow matmuls pair two 128-contraction tiles per
    # instruction at 0.5 cycles/row: the 3-term (hh + lh + hl) projection
    # runs at ~3/4 the PE cost of bf16. Layout contiguous in s so streamed
    # 512-token slices have 512B runs per partition (no DMA penalty).
    xh = nc.dram_tensor("xh", [P, DD, SEQ], e4, kind="ExternalInput").ap()
    xl = nc.dram_tensor("xl", [P, DD, SEQ], e4, kind="ExternalInput").ap()
    # packed projection weights [wq_c; wk_c; wv_c]: w?[p, t, f] ~ w[f, 128t+p]
    wh = nc.dram_tensor("wh", [P, DD, EW + 2 * D], e4, kind="ExternalInput").ap()
    wl = nc.dram_tensor("wl", [P, DD, EW + 2 * D], e4, kind="ExternalInput").ap()
    # rope tables [cos_rep | sin_rep] with sqrt(scale) folded in
    cs = nc.dram_tensor("cs", [SEQ, EW], bf16, kind="ExternalInput").ap()
    # transposed partial mask blocks [k 128, q <=512], left-aligned;
    # bf16 0/1 factors when the mask is multiplicative, else f32 additive
    mul_mask = any(
        e[4] for row in plan for e in row if e[1] >= 0
    )
    mb = nc.dram_tensor(
        "maskb", [max(n_uniq, 1), P, 512], bf16 if mul_mask else f32,
        kind="ExternalInput",
    ).ap()
    # wo transposed, 64x-scaled, fp8 hi/lo split, pre-tiled [p, mc, jt, m] so
    # each 256-col chunk is one DMA with 8KB-contiguous per-partition runs
    MC2 = DIM // 256
    woh = nc.dram_tensor(
        "woh", [P, MC2, JT, 256], e4, kind="ExternalInput"
    ).ap()
    wol = nc.dram_tensor(
        "wol", [P, MC2, JT, 256], e4, kind="ExternalInput"
    ).ap()
    out = nc.dram_tensor("out", [NH * 64, DIM], bf16, kind="ExternalOutput").ap()

    with tile.TileContext(nc) as tc, ExitStack() as ctx:
        const = ctx.enter_context(tc.tile_pool(name="const", bufs=1))
        idP = const.tile([P, P], bf16)
        make_identity(nc, idP)
        # 1/16 folds the fp8 A-staging scale into the softmax denominators:
        # dpt = sum(P)/16 -> recip = 16/sum -> Aall holds 16*A (fp8-friendly)
        ones = const.tile([P, 1], bf16)
        nc.vector.memset(ones, 1.0 / 16.0)
        # 1/1024 row for DVE-side output descaling (plain TT is walrus-safe)
        sc1024 = const.tile([P, 256], f32)
        nc.vector.memset(sc1024, 1.0 / 1024.0)

        pers = ctx.enter_context(tc.tile_pool(name="pers", bufs=1))
        QT = pers.tile([P, NH, SEQ], bf16)   # [d, h, s]
        KT = pers.tile([P, SEQ], bf16)       # [d, s]
        Vt = pers.tile([P, ST, D], bf16)     # [s(part), stile, d]
        if n_uniq > 0:
            mbt = pers.tile([P, n_uniq, 512], bf16 if mul_mask else f32)
        Aall = [
            pers.tile([P, 2 * ST * D], bf16, name=f"Aall{i}")
            for i in range(ITILES)
        ]
        A8h = [
            pers.tile([P, 2 * ST * D], e4, name=f"A8h{i}")
            for i in range(ITILES)
        ]
        A8l = [
            pers.tile([P, 2 * ST * D], e4, name=f"A8l{i}")
            for i in range(ITILES)
        ]
        stat = ctx.enter_context(tc.tile_pool(name="stat", bufs=16))

        # ---------------- phase 1 ----------------
        with (
            tc.tile_pool(name="wpool", bufs=1) as wpool,
            tc.tile_pool(name="xpool", bufs=2) as xpool,
            tc.tile_pool(name="cspool", bufs=4) as cspool,
            tc.tile_pool(name="rpool", bufs=2) as rpool,
            tc.tile_pool(name="qps", bufs=2, space="PSUM") as qps,
            tc.tile_pool(name="kvps", bufs=2, space="PSUM") as kvps,
            tc.tile_pool(name="tps", bufs=2, space="PSUM") as tps,
        ):
            # startup-latency-aware DMA order: the first (hi*hi) matmuls need
            # only wh t-tiles 0..3 and the first half of xh s-group 0; the
            # hi*lo and lo*hi terms follow, so wl/xl stream behind
            wht = wpool.tile([P, DD, EW + 2 * D], e4, name="wht")
            wlt = wpool.tile([P, DD, EW + 2 * D], e4, name="wlt")
            xs0h = xpool.tile([P, DD, 512], e4, tag="xsh", name="xs0h")
            xs0l = xpool.tile([P, DD, 512], e4, tag="xsl", name="xs0l")
            # supply order tracks the term order hh -> hl -> lh of s-group 0,
            # in fine-grained chunks so the first matmuls start after ~2 DMAs
            cst0 = [cspool.tile([P, EW], bf16, tag="cs", name=f"cst{l}")
                    for l in range(4)]
            for g in range(8):
                nc.sync.dma_start(
                    out=wht[:, 4 * g : 4 * g + 4, :],
                    in_=wh[:, 4 * g : 4 * g + 4, :],
                )
                nc.sync.dma_start(
                    out=xs0h[:, 4 * g : 4 * g + 4, :],
                    in_=xh[:, 4 * g : 4 * g + 4, 0:512],
                )
                if g == 1:
                    for l in range(2):
                        nc.sync.dma_start(
                            out=cst0[l], in_=cs[l * P : (l + 1) * P, :]
                        )
            for g in range(8):
                nc.sync.dma_start(
                    out=wlt[:, 4 * g : 4 * g + 4, :],
                    in_=wl[:, 4 * g : 4 * g + 4, :],
                )
                nc.sync.dma_start(
                    out=xs0l[:, 4 * g : 4 * g + 4, :],
                    in_=xl[:, 4 * g : 4 * g + 4, 0:512],
                )
                if g == 1:
                    for l in range(2, 4):
                        nc.sync.dma_start(
                            out=cst0[l], in_=cs[l * P : (l + 1) * P, :]
                        )
            def finish_stile(st, cst, Qp, KVp):
                # rope via strided even/odd halves; final add/sub writes
                # bf16 so the PE transposes below run at 1 cycle/row
                HF = EW // 2
                rq = rpool.tile([P, EW], bf16, tag="rq")
                rk = rpool.tile([P, D], bf16, tag="rk")
                t1 = rpool.tile([P, HF], f32, tag="t1")
                t2 = rpool.tile([P, HF], f32, tag="t2")
                q_ev, q_od = Qp[:, 0:EW:2], Qp[:, 1:EW:2]
                cosr, sinr = cst[:, 0:HF], cst[:, HF : 2 * HF]
                nc.vector.tensor_tensor(out=t1, in0=q_ev, in1=cosr, op=A_.mult)
                nc.vector.tensor_tensor(out=t2, in0=q_od, in1=sinr, op=A_.mult)
                nc.vector.tensor_tensor(
                    out=rq[:, 0:EW:2], in0=t1, in1=t2, op=A_.subtract
                )
                nc.vector.tensor_tensor(out=t1, in0=q_ev, in1=sinr, op=A_.mult)
                nc.vector.tensor_tensor(out=t2, in0=q_od, in1=cosr, op=A_.mult)
                nc.vector.tensor_tensor(
                    out=rq[:, 1:EW:2], in0=t1, in1=t2, op=A_.add
                )

                k_ev, k_od = KVp[:, 0:D:2], KVp[:, 1:D:2]
                cosk, sink = cst[:, 0 : D // 2], cst[:, HF : HF + D // 2]
                t1k, t2k = t1[:, 0 : D // 2], t2[:, 0 : D // 2]
                nc.vector.tensor_tensor(out=t1k, in0=k_ev, in1=cosk, op=A_.mult)
                nc.vector.tensor_tensor(out=t2k, in0=k_od, in1=sink, op=A_.mult)
                nc.vector.tensor_tensor(
                    out=rk[:, 0:D:2], in0=t1k, in1=t2k, op=A_.subtract
                )
                nc.vector.tensor_tensor(out=t1k, in0=k_ev, in1=sink, op=A_.mult)
                nc.vector.tensor_tensor(out=t2k, in0=k_od, in1=cosk, op=A_.mult)
                nc.vector.tensor_tensor(
                    out=rk[:, 1:D:2], in0=t1k, in1=t2k, op=A_.add
                )

                # V -> bf16 [s, d] (ACT copy, cast; 1/1024 undoes the
                # fp8 staging scales 16x on x and 64x on w)
                nc.scalar.activation(
                    out=Vt[:, st, :], in_=KVp[:, D : 2 * D], func=AF.Copy,
                    scale=1.0 / 1024.0,
                )

                # transpose rq (per head) and rk into bf16 [d, s] layouts
                Tt = tps.tile([P, EW + D], bf16, tag="T")
                for h in range(NH):
                    nc.tensor.transpose(
                        Tt[:, h * P : (h + 1) * P], rq[:, h * P : (h + 1) * P],
                        idP,
                    )
                nc.tensor.transpose(Tt[:, EW : EW + D], rk, idP)
                nc.scalar.activation(
                    out=QT[:, :, st * P : (st + 1) * P],
                    in_=Tt[:, 0:EW].rearrange("p (h s) -> p h s", h=NH),
                    func=AF.Copy,
                )
                nc.scalar.activation(
                    out=KT[:, st * P : (st + 1) * P],
                    in_=Tt[:, EW : EW + D],
                    func=AF.Copy,
                )

            SG = ST // 4  # stream x in 4 groups of 4 s-tiles
            for sg in range(SG):
                if sg == 0:
                    xsh, xsl = xs0h, xs0l
                    csts = cst0
                else:
                    xsh = xpool.tile([P, DD, 512], e4, tag="xsh")
                    nc.sync.dma_start(
                        out=xsh, in_=xh[:, :, sg * 512 : (sg + 1) * 512]
                    )
                    xsl = xpool.tile([P, DD, 512], e4, tag="xsl")
                    nc.sync.dma_start(
                        out=xsl, in_=xl[:, :, sg * 512 : (sg + 1) * 512]
                    )
                    csts = []
                    for l in range(4):
                        st = 4 * sg + l
                        c = cspool.tile([P, EW], bf16, tag="cs")
                        nc.sync.dma_start(
                            out=c, in_=cs[st * P : (st + 1) * P, :]
                        )
                        csts.append(c)
                TP = DD // 2
                terms = [(xsh, wht), (xsh, wlt), (xsl, wht)]

                def emit_mm(ls, tiles):
                    # emit the 3-term DR chains for s-tiles `ls`, interleaved
                    # tp-major so demand tracks the chunked DMA supply order;
                    # each s-tile's rope/store is emitted as soon as ITS
                    # chain closes so the psum bufs recycle early
                    for term, (xst, wt) in enumerate(terms):
                        for tp in range(TP):
                            for i, l in enumerate(ls):
                                Qp, KVp = tiles[i]
                                lhsT = xst[:, 2 * tp : 2 * tp + 2,
                                           l * P : (l + 1) * P]
                                first = term == 0 and tp == 0
                                last = term == 2 and tp == TP - 1
                                nc.tensor.matmul(
                                    Qp, lhsT,
                                    wt[:, 2 * tp : 2 * tp + 2, 0:EW],
                                    start=first, stop=last, perf_mode=DR,
                                )
                                nc.tensor.matmul(
                                    KVp, lhsT,
                                    wt[:, 2 * tp : 2 * tp + 2,
                                       EW : EW + 2 * D],
                                    start=first, stop=last, perf_mode=DR,
                                )
                            if term == 2 and tp == TP - 1:
                                for i, l in enumerate(ls):
                                    finish_stile(4 * sg + l, csts[l],
                                                 *tiles[i])

                for ls in [(0,), (1,), (2,), (3,)]:
                    tiles = [
                        (qps.tile([P, EW], f32, tag="Qp", name=f"Qp{l}"),
                         kvps.tile([P, 2 * D], f32, tag="KVp", name=f"KVp{l}"))
                        for l in ls
                    ]
                    emit_mm(ls, tiles)
        # ---------------- phase 2: head-pair blocks ----------------
        # per unit (key tile kt): scores for BOTH heads of the pair into one
        # [P, 1024] psum tile, ONE fused exp over both halves, deferred PV.
        # Each block's store segment (recips / A store) is emitted LAGGED,
        # after the next block's first unit, so the ACT exp stream never
        # breaks at block boundaries.
        wopool = ctx.enter_context(tc.tile_pool(name="wopool", bufs=6))
        osb = ctx.enter_context(tc.tile_pool(name="osb", bufs=2))
        with (
            tc.tile_pool(name="ptpool", bufs=20) as ptpool,
            tc.tile_pool(name="atsb", bufs=4) as atsb,
            tc.tile_pool(name="sps", bufs=2, space="PSUM") as sps,
            tc.tile_pool(name="atps", bufs=2, space="PSUM") as atps,
            # one [P, 1024] bf16 bank carries BOTH the Ap transposes (cols
            # 0:512) and, bitcast to f32, the denominator accumulators (cols
            # 512:528) -- PSUM slots are bank-granular, and packing them
            # frees the 8th bank for the ops_inj pool below
            tc.tile_pool(name="aps", bufs=1, space="PSUM") as aps,
            tc.tile_pool(name="ops_inj", bufs=1, space="PSUM") as ops_inj,
        ):
            # wo prefetch (SP queue position: right after phase-1 DMAs; the
            # DMA device is idle during phase 2). 256-wide chunks, hi+lo fp8.
            wots = {}

            def stage_wo(mc):
                th = wopool.tile([P, JT, 256], e4, tag="woh", name=f"woh{mc}")
                tl = wopool.tile([P, JT, 256], e4, tag="wol", name=f"wol{mc}")
                nc.sync.dma_start(out=th, in_=woh[:, mc, :, :])
                nc.sync.dma_start(out=tl, in_=wol[:, mc, :, :])
                wots[mc] = (th, tl)

            if n_uniq > 0:
                nc.sync.dma_start(out=mbt, in_=mb.rearrange("u p m -> p u m"))
            for mc in range(6):
                stage_wo(mc)

            def pair_block_gen(hp, qs):
                """hp in {0,1}: heads (2hp, 2hp+1). Yields per unit; the
                final segment (after last yield) is the block tail."""
                row = plan[qs]
                h0 = 2 * hp
                if not row:
                    for qi in range(4):
                        i = 4 * qs + qi
                        nc.vector.memset(
                            Aall[hp][:, i * 2 * P : (i + 1) * 2 * P], 0.0
                        )
                    yield
                    return
                Ats = [atps.tile([P, 512], f32, tag="At", name=f"At{hl}")
                       for hl in range(2)]
                mt = aps.tile([P, 1024], bf16, tag="Ap")
                Ap = mt[:, 0:512]
                dpt = mt[:, 512:528].bitcast(f32)  # [P, 8] f32 region
                pts = []
                pend = None  # deferred PV (one unit of lag hides exp latency)
                for n, (kt, uid, lead, w, mul) in enumerate(row):
                    if pend is not None:
                        pkt, pPT, pn = pend
                        for hl in range(2):
                            nc.tensor.matmul(
                                Ats[hl], Vt[:, pkt, :],
                                pPT[:, hl * 512 : (hl + 1) * 512],
                                start=(pn == 0), stop=False,
                            )
                    off = lead * P
                    S = sps.tile([P, 1024], f32, tag="S")
                    for hl in range(2):
                        nc.tensor.matmul(
                            S[:, hl * 512 + off : (hl + 1) * 512],
                            KT[:, kt * P : (kt + 1) * P],
                            QT[:, h0 + hl, qs * 512 + off : (qs + 1) * 512],
                            start=True, stop=True,
                        )
                        if uid >= 0 and not mul:
                            # additive mask must run pre-exp (ACT waits on it)
                            sl = S[:, hl * 512 + off : hl * 512 + off + w]
                            nc.vector.tensor_add(sl, sl, mbt[:, uid, 0:w])
                    PT = ptpool.tile([P, 1024], bf16, tag="PT")
                    sv = S.rearrange("p (hl q) -> p hl q", hl=2)[:, :, off:512]
                    pv = PT.rearrange("p (hl q) -> p hl q", hl=2)[:, :, off:512]
                    nc.scalar.activation(out=pv, in_=sv, func=AF.Exp)
                    if uid >= 0 and mul:
                        # 0/1 mask applied to P post-exp (all-bf16 -> DVE 2x
                        # mode; the deferred PV gives it a unit of slack)
                        for hl in range(2):
                            sl = PT[:, hl * 512 + off : hl * 512 + off + w]
                            nc.vector.tensor_tensor(
                                out=sl, in0=sl, in1=mbt[:, uid, 0:w],
                                op=A_.mult,
                            )
                    if off:
                        for hl in range(2):
                            nc.vector.memset(
                                PT[:, hl * 512 : hl * 512 + off], 0.0
                            )
                    pts.append(PT)
                    pend = (kt, PT, n)
                    yield
                pkt, pPT, pn = pend
                for hl in range(2):
                    nc.tensor.matmul(
                        Ats[hl], Vt[:, pkt, :],
                        pPT[:, hl * 512 : (hl + 1) * 512],
                        start=(pn == 0), stop=True,
                    )
                # denominators via 1-wide ones-matmuls (the interp models one
                # live accumulation group per PSUM bank, so the 8 chains run
                # one after another). Emitted in-block; the store segment
                # below is lagged into the next block.
                for hl in range(2):
                    for qi in range(4):
                        col = hl * 4 + qi
                        for n, PT in enumerate(pts):
                            nc.tensor.matmul(
                                dpt[:, col : col + 1],
                                PT[:, hl * 512 + qi * P : hl * 512 + (qi + 1) * P],
                                ones,
                                start=(n == 0), stop=(n == len(pts) - 1),
                                skip_group_check=True,
                            )
                yield  # --- lag point: store segment pulled later ---
                for hl in range(2):
                    recips = []
                    for qi in range(4):
                        col = hl * 4 + qi
                        r = stat.tile([P, 1], f32, tag="recip")
                        nc.vector.reciprocal(r, dpt[:, col : col + 1])
                        recips.append(r)
                    Atsb = atsb.tile([P, 512], bf16, tag="Atsb")
                    nc.vector.tensor_copy(out=Atsb, in_=Ats[hl])
                    for qi in range(4):
                        nc.tensor.transpose(
                            Ap[:, qi * P : (qi + 1) * P],
                            Atsb[:, qi * P : (qi + 1) * P],
                            idP,
                        )
                    # Aall layout: [sp, (t*2 + dd)*128 + hb*64 + p] so the
                    # final matmul's stationary slices are contiguous
                    Ah = Aall[hp]
                    for qi in range(4):
                        i = 4 * qs + qi
                        dview = Ah[:, i * 2 * P : (i + 1) * 2 * P].rearrange(
                            "a (dd j) -> a dd j", dd=2
                        )[:, :, hl * 64 : hl * 64 + 64].rearrange(
                            "a dd p -> a p dd"
                        )
                        src = Ap[:, qi * P : (qi + 1) * P].rearrange(
                            "a (p two) -> a p two", two=2
                        )
                        # per-partition scale-copy on ACT (walrus rejects the
                        # DVE InstTensorScalarPtr equivalent)
                        nc.scalar.activation(
                            out=dview, in_=src, func=AF.Copy,
                            scale=recips[qi],
                        )

            def p3_gen(it, mc, opool, on_act):
                """Yield after each of the 3 DR terms so injection can feed
                the PE in ~0.9us segments without starving the ACT exps."""
                woth, wotl = wots[mc]
                O = opool.tile([P, 256], f32, tag="O")
                # 3-term fp8 DoubleRow: (Ah+Al)@(wh+wl) minus the Al@wl
                # term; each DR instruction contracts a PAIR of j-tiles
                terms = [(A8h[it], woth), (A8h[it], wotl), (A8l[it], woth)]
                for term, (Av, wv) in enumerate(terms):
                    Av4 = Av.rearrange("p (t d j) -> p t d j", d=2, j=P)
                    for ddj in range(2):
                        for u in range(ST // 2):
                            first = term == 0 and ddj == 0 and u == 0
                            last = (term == 2 and ddj == 1
                                    and u == ST // 2 - 1)
                            nc.tensor.matmul(
                                O,
                                Av4[:, 2 * u : 2 * u + 2, ddj, :],
                                wv[:, ddj * ST + 2 * u
                                   : ddj * ST + 2 * u + 2, :],
                                start=first, stop=last, perf_mode=DR,
                            )
                    if term < 2:
                        yield
                Ot = osb.tile([P, 256], bf16, tag="Ot")
                # 1/1024 undoes the 16x (A) and 64x (wo) fp8 scales; during
                # phase 2 the evacuation runs on DVE (ACT is the softmax
                # bottleneck), in the tail on ACT (SP busy streaming wo)
                if on_act:
                    nc.scalar.activation(
                        out=Ot, in_=O, func=AF.Copy, scale=1.0 / 1024.0
                    )
                    nc.scalar.dma_start(
                        out=out[it * P : (it + 1) * P,
                                mc * 256 : (mc + 1) * 256],
                        in_=Ot,
                    )
                else:
                    nc.vector.tensor_tensor(
                        out=Ot, in0=O, in1=sc1024, op=A_.mult
                    )
                    nc.sync.dma_start(
                        out=out[it * P : (it + 1) * P,
                                mc * 256 : (mc + 1) * 256],
                        in_=Ot,
                    )

            def p3_block(it, mc, opool, on_act):
                for _ in p3_gen(it, mc, opool, on_act):
                    pass

            def split_a8(it, chunks=4, on_act=True):
                # split 16*A (bf16) into fp8 hi + residual
                for ch in range(chunks):
                    sl = slice(ch * (2 * ST * D // chunks),
                               (ch + 1) * (2 * ST * D // chunks))
                    if on_act:
                        nc.scalar.activation(
                            out=A8h[it][:, sl], in_=Aall[it][:, sl],
                            func=AF.Copy,
                        )
                    else:
                        nc.vector.tensor_copy(
                            out=A8h[it][:, sl], in_=Aall[it][:, sl]
                        )
                    nc.vector.tensor_tensor(
                        out=A8l[it][:, sl], in0=Aall[it][:, sl],
                        in1=A8h[it][:, sl], op=A_.subtract,
                    )

            # drive blocks hp-major; pull the previous block's store segment
            # right after the current block's FIRST unit (before the current
            # block's first PV touches the recycled At banks). Once hp0's A
            # is complete, early it0 output-projection blocks are injected
            # one DR TERM per softmax unit: the PE fills its idle time with
            # dense work while the ACT exp stream stays fed, all injected
            # evacuation off the ACT critical path. Consumed wo ring slots
            # are immediately re-staged on the idle phase-2 DMA device.
            N_INJ = 6
            inj = {"gen": None, "mc": 0, "stage": [0, 1, 2, 3, 4, 5]}

            def pump_inj():
                if inj["gen"] is None:
                    if inj["mc"] >= N_INJ:
                        return
                    inj["gen"] = p3_gen(0, inj["mc"], ops_inj, on_act=False)
                try:
                    next(inj["gen"])
                except StopIteration:
                    inj["gen"] = None
                    inj["mc"] += 1
                    if inj["stage"]:
                        stage_wo(inj["stage"].pop(0))

            pending = None
            for hp in range(2):
                for qs in range(QS):
                    g = pair_block_gen(hp, qs)
                    L = len(plan[qs])
                    nunits = max(1, L)
                    for n in range(nunits + (1 if L else 0)):
                        next(g)  # units, then the denominator segment
                        if n == 0 and pending is not None:
                            try:
                                next(pending)
                            except StopIteration:
                                pass
                            pending = None
                            if hp == 1 and qs == 0:
                                # hp0's last store just emitted: A of row
                                # tile 0 is complete -> split it on DVE
                                split_a8(0, on_act=False)
                    if pending is not None:  # empty-row block: flush now
                        try:
                            next(pending)
                        except StopIteration:
                            pass
                    pending = g if L else None
                    if hp == 1 and qs >= 1:
                        npump = {1: 4, 2: 6, 3: 8}[qs]
                        for _ in range(npump):
                            pump_inj()
            if pending is not None:
                try:
                    next(pending)
                except StopIteration:
                    pass
            # it1's A is now complete: split it on DVE under the remaining
            # injected blocks so the tail starts unblocked
            split_a8(1, on_act=False)
            while inj["mc"] < N_INJ:
                pump_inj()

        # ---------------- phase 3 tail ----------------
        # chunks 6..9 were consumed by injected it0 blocks and not yet
        # re-staged; chunks 10..15 get their first staging inside the loop
        for mc in range(6, N_INJ):
            stage_wo(mc)
        with tc.tile_pool(name="ops", bufs=4, space="PSUM") as ops:
            for mc in range(MC2):
                if N_INJ <= mc + 6 < MC2:
                    stage_wo(mc + 6)
                for it in range(ITILES):
                    if it == 0 and mc < N_INJ:
                        continue  # done during phase 2
                    p3_block(it, mc, ops, on_act=True)

    nc.compile()
    return nc


def analyze_mask(mask, SEQ):
    """Classify transposed 128-key x 512-query mask blocks per (kt, qs):
    skip / free / masked(dedup uid). Partial blocks store only the masked
    window: lead = # of leading fully-masked 128-col spans, w = width of the
    remaining span that contains any masked element. Blocks stored TRANSPOSED
    [k, q], left-aligned into a [128, 512] buffer."""
    QS = SEQ // 512
    KTOT = SEQ // P
    uniq = {}
    blocks = []
    plan = []
    # every query row needs at least one allowed key (no-max softmax would
    # otherwise divide by zero; the reference's uniform-distribution quirk
    # for fully-masked rows is not representable in this fast path)
    assert (mask > NEG_THRESH).any(axis=1).all(), "fully masked query row"
    # a pure 0/-inf mask can be applied MULTIPLICATIVELY (0/1) to P after the
    # exp, off the matmul->exp critical path; finite-valued masks must stay
    # additive pre-exp
    mul = bool(((mask <= NEG_THRESH) | (mask == 0.0)).all())
    for qs in range(QS):
        row = []
        for kt in range(KTOT):
            blk = mask[qs * 512 : (qs + 1) * 512, kt * P : (kt + 1) * P]
            if (blk <= NEG_THRESH).all():
                continue
            if not blk.any():
                row.append((kt, -1, 0, 0, mul))
                continue
            bT = np.ascontiguousarray(blk.T)  # [128 k, 512 q]
            col_all = (bT <= NEG_THRESH).all(axis=0)
            col_any = bT.any(axis=0)
            lead = 0
            while lead < 3 and col_all[lead * P : (lead + 1) * P].all():
                lead += 1
            last_any = int(np.nonzero(col_any)[0].max())
            w = (last_any // P + 1) * P - lead * P
            sl = bT[:, lead * P : lead * P + w]
            key = (w, sl.tobytes())
            if key not in uniq:
                uniq[key] = len(blocks)
                buf = np.zeros((P, 512), np.float32)
                buf[:, 0:w] = (sl > NEG_THRESH) if mul else sl
                blocks.append(buf)
            row.append((kt, uniq[key], lead, w, mul))
        plan.append(row)
    return plan, blocks


def make_rope_tables(cos_freq, sin_freq, SEQ, scale_quarter):
    """[cos_rep (SEQ, NH*64) | sin_rep (SEQ, NH*64)], sqrt(scale) folded in."""
    cos_t = np.tile(np.asarray(cos_freq, np.float32) * scale_quarter, (1, NH))
    sin_t = np.tile(np.asarray(sin_freq, np.float32) * scale_quarter, (1, NH))
    import ml_dtypes
    return np.ascontiguousarray(
        np.concatenate([cos_t, sin_t], axis=1).astype(ml_dtypes.bfloat16)
    )


def stage_in_maps(x, cos_freq, sin_freq, wq, wk, wv, wo, plan, blocks):
    """Host-side input staging (shared by kernel() and test harnesses)."""
    import ml_dtypes

    bf16 = ml_dtypes.bfloat16
    e4 = ml_dtypes.float8_e4m3
    SEQ, DIM = x.shape
    DD = DIM // P
    n_uniq = len(blocks)
    # rope tables fold sqrt(scale) per side AND 1/1024 (fp8 staging scales)
    scale_quarter = np.float32(D ** -0.25) / np.float32(1024.0)
    cs = make_rope_tables(cos_freq, sin_freq, SEQ, scale_quarter)

    def tile_x(a):
        # [p, t, s] = a[s, 128t+p]
        return np.ascontiguousarray(a.reshape(SEQ, DD, P).transpose(2, 1, 0))

    xs16 = 16.0 * np.asarray(x, np.float32)
    xh8 = xs16.astype(e4)
    xl8 = (xs16 - xh8.astype(np.float32)).astype(e4)
    xh = tile_x(xh8)
    xl = tile_x(xl8)
    # wo: transpose, 64x scale, fp8 hi/lo, tile [p, mc, jt, m]
    JT, MC2 = 2 * SEQ // P, DIM // 256
    ws64 = 64.0 * np.asarray(wo, np.float32).T
    woh8 = ws64.astype(e4)
    wol8 = (ws64 - woh8.astype(np.float32)).astype(e4)

    def tile_wo(a):
        return np.ascontiguousarray(
            a.reshape(JT, P, MC2, 256).transpose(1, 2, 0, 3)
        )

    woh = tile_wo(woh8)
    wol = tile_wo(wol8)
    mul_mask = any(e[4] for row in plan for e in row if e[1] >= 0)
    mb_dt = bf16 if mul_mask else np.float32
    if n_uniq:
        mbs = np.ascontiguousarray(np.stack(blocks, axis=0)).astype(mb_dt)
    else:
        mbs = np.zeros((1, P, 512), mb_dt)

    in_maps = []
    for c in range(CORES):
        w_c = np.concatenate(
            [
                wq[c * NH * D : (c + 1) * NH * D],
                wk[c * D : (c + 1) * D],
                wv[c * D : (c + 1) * D],
            ],
            axis=0,
        ).astype(np.float32)  # (768, DIM)
        # w?[p, t, f] ~ w_c[f, 128t+p], scaled 64x and split hi/lo in fp8
        ws64 = (64.0 * w_c.T).reshape(DD, P, 768).transpose(1, 0, 2)
        wh8 = ws64.astype(e4)
        wl8 = (ws64 - wh8.astype(np.float32)).astype(e4)
        in_maps.append(
            {
                "xh": xh, "xl": xl,
                "wh": np.ascontiguousarray(wh8),
                "wl": np.ascontiguousarray(wl8),
                "cs": cs, "maskb": mbs, "woh": woh, "wol": wol,
            }
        )
    return in_maps


_BUILD_CACHE = {}


def kernel(
    x,
    cos_freq,
    sin_freq,
    positions,
    mask,
    wq,
    wk,
    wv,
    wo,
    _trace=False,
):
    import sys

    if "/opt/trn_rl_repo" not in sys.path:
        sys.path.insert(0, "/opt/trn_rl_repo")
    from concourse.bass_utils import run_bass_kernel_spmd

    x = np.asarray(x, np.float32)
    mask = np.asarray(mask, np.float32)
    wq = np.asarray(wq, np.float32)
    wk = np.asarray(wk, np.float32)
    wv = np.asarray(wv, np.float32)
    wo = np.asarray(wo, np.float32)
    SEQ, DIM = x.shape
    assert wq.shape[0] == CORES * NH * D and wk.shape[0] == CORES * D
    assert 2 * SEQ == wq.shape[0], "flatten structure requires H*D == 2*SEQ"

    plan, blocks = analyze_mask(mask, SEQ)
    n_uniq = len(blocks)
    key = (SEQ, DIM, tuple(tuple(r) for r in plan))
    if key not in _BUILD_CACHE:
        _BUILD_CACHE[key] = build_attention_nc(SEQ, DIM, plan, n_uniq)
    nc = _BUILD_CACHE[key]

    in_maps = stage_in_maps(
        x, cos_freq, sin_freq, wq, wk, wv, wo, plan, blocks
    )

    import time as _time

    _t0 = _time.time()
    res = run_bass_kernel_spmd(nc, in_maps, list(range(CORES)), trace=_trace)
    global LAST_EXEC_NS
    LAST_EXEC_NS = int((_time.time() - _t0) * 1e9)
    outp = np.concatenate(
        [res.results[c]["out"] for c in range(CORES)], axis=0
    ).astype(np.float32)
    if _trace:
        return outp, res
    return outp
